# revision 1
# baseline (speedup 1.0000x reference)
"""GATv2 (2-layer, 2-head) Trainium2 kernel, 8-core SPMD — v2.

vs baseline: own-shard tables + AllGather (no replicated full-table compute),
xr[dst] per edge split between DMA gather (subtiles 0..6) and PE one-hot
broadcast (subtiles 7..13) built from a ones-matmul row-broadcast + DVE
is_equal, batch-merged DVE ops, finalize on ACT with per-head scale and the
1/att + bias fused into a post-transpose Relu, L2 local tables inlined into
L1 finalize (no h1T round-trip), gathers split across 6 SWDGE queues.
"""
import sys

sys.path.insert(0, "/opt/trn_rl_repo")

import numpy as np
import ml_dtypes

BF = ml_dtypes.bfloat16

# ---- static layout constants (match reference problem sizes) ----
N = 50000
NCORES = 8
LANES = 128
NTILES = 49
SPC = NTILES * LANES          # 6272 slots per core
S = NCORES * SPC              # 50176 total slots
HALF = S // 2                 # 25088
TA = 7                        # half-A gather subtiles per dst-tile
TB = 7
TS = TA + TB                  # random-edge subtiles (self subtile is extra)
XG = 7                        # subtiles with xr via DMA gather (rest via PE)
XP = TS - XG                  # subtiles with xr via PE broadcast
GB = 3                        # dst-tiles per gather batch
IN_F = 128
HC = 256                      # H*C
OUT_F = 40
SLOPE = 0.2
# AllGather chunking: two tile groups (sum = NTILES). The gather table is
# laid out chunk-major: [chunk, core, tile-in-chunk, lane]. The chunk
# boundary doubles as the int16 gather-table A/B split.
AG_CH = (24, 25)
AG_T0 = (0, 24)               # first tile of each chunk
AG_BASE = (0, 24 * 128 * NCORES)
HALFR = AG_BASE[1]            # 24576: gather tables A=[0,HALFR) B=[HALFR,S)

_NC_CACHE = {}
_RUN_OPTS = {}
_LAST_RESULTS = {}
_LR_RANGES = {}


# ---------------------------------------------------------------- host prep
def _pack_graph(src, dst):
    deg = np.bincount(dst, minlength=N)

    is_self = src == dst
    self_eids = np.full(N, -1, np.int64)
    sids = np.where(is_self)[0]
    self_eids[src[sids]] = sids
    rand_mask = np.ones(len(src), bool)
    rand_mask[self_eids[self_eids >= 0]] = False

    nodes_per_core = (N + NCORES - 1) // NCORES
    order = np.argsort(-deg, kind="stable")
    core_edges = np.zeros(NCORES, np.int64)
    core_nodes = np.zeros(NCORES, np.int64)
    core_of_node = np.full(N, -1, np.int32)
    for v in order:
        k = np.argmin(np.where(core_nodes < nodes_per_core, core_edges, 1 << 60))
        core_of_node[v] = k
        core_edges[k] += deg[v]
        core_nodes[k] += 1

    rsrc, rdst = src[rand_mask], dst[rand_mask]

    # --- chunk-group assignment per core (before tile packing): deal nodes
    # round-robin by out-degree so A/B table halves see balanced sources.
    NG = len(AG_CH)
    odeg = np.bincount(rsrc, minlength=N)
    group_of_node = np.full(N, -1, np.int8)
    gcap = [c * LANES for c in AG_CH]
    for k in range(NCORES):
        vs = np.where(core_of_node == k)[0]
        vs = vs[np.argsort(-odeg[vs], kind="stable")]
        cnt = [0] * NG
        gi = 0
        for v in vs:
            while cnt[gi % NG] >= gcap[gi % NG]:
                gi += 1
            group_of_node[v] = gi % NG
            cnt[gi % NG] += 1
            gi += 1
    eh_node = (group_of_node >= 1).astype(np.int8)

    dA = np.bincount(rdst[eh_node[rsrc] == 0], minlength=N)
    dB = np.bincount(rdst[eh_node[rsrc] == 1], minlength=N)
    capA, capB = TA * LANES, TB * LANES

    tile_of_node = np.full(N, -1, np.int32)
    lane_of_node = np.full(N, -1, np.int32)
    for k in range(NCORES):
        for g in range(NG):
            vs = np.where((core_of_node == k) & (group_of_node == g))[0]
            vs = vs[np.argsort(-(dA[vs] + dB[vs]), kind="stable")]
            nv = len(vs)
            ntg = AG_CH[g]
            tile = np.empty(nv, np.int64)
            for i in range(nv):
                r, c = divmod(i, ntg)
                tile[i] = c if r % 2 == 0 else ntg - 1 - c
            loadA = np.bincount(tile, weights=dA[vs],
                                minlength=ntg).astype(np.int64)
            loadB = np.bincount(tile, weights=dB[vs],
                                minlength=ntg).astype(np.int64)
            it = 0
            while (loadA.max() > capA or loadB.max() > capB) and it < 100000:
                it += 1
                t_bad = int(np.argmax(np.maximum(loadA - capA, loadB - capB)))
                overA = loadA[t_bad] - capA >= loadB[t_bad] - capB
                t_good = int(np.argmin(loadA + loadB))
                in_bad = np.where(tile == t_bad)[0]
                in_good = np.where(tile == t_good)[0]
                d_bad = dA[vs[in_bad]] if overA else dB[vs[in_bad]]
                ib = in_bad[np.argmax(d_bad)]
                ig = in_good[np.argmin(dA[vs[in_good]] + dB[vs[in_good]])]
                for i, frm, to in ((ib, t_bad, t_good), (ig, t_good, t_bad)):
                    v = vs[i]
                    tile[i] = to
                    loadA[frm] -= dA[v]; loadA[to] += dA[v]
                    loadB[frm] -= dB[v]; loadB[to] += dB[v]
            if loadA.max() > capA or loadB.max() > capB:
                raise RuntimeError("edge packing failed; need bigger TA/TB")
            tile_of_node[vs] = AG_T0[g] + tile
            for t in range(ntg):
                nodes_t = vs[tile == t]
                lane_of_node[nodes_t] = np.arange(len(nodes_t))

    slot_of_node = (core_of_node.astype(np.int64) * SPC
                    + tile_of_node * LANES + lane_of_node)
    node_of_slot = np.full(S, -1, np.int64)
    node_of_slot[slot_of_node] = np.arange(N)

    # chunk-major gather-table row of each node
    g_arr = group_of_node.astype(np.int64)
    base = np.asarray(AG_BASE, np.int64)[g_arr]
    t0 = np.asarray(AG_T0, np.int64)[g_arr]
    chw = np.asarray(AG_CH, np.int64)[g_arr]
    grow_of_node = (base + core_of_node * chw * LANES
                    + (tile_of_node - t0) * LANES + lane_of_node)

    srcrow = grow_of_node[rsrc]
    dstslot = slot_of_node[rdst]
    dst_core = (dstslot // SPC).astype(np.int32)
    dst_tile = ((dstslot % SPC) // LANES).astype(np.int32)
    dst_lane = (dstslot % LANES).astype(np.int32)
    eh = (srcrow >= HALFR).astype(np.int8)

    idxXL = np.zeros((NCORES, NTILES, TS * 128), np.int16)
    idxXR = np.zeros((NCORES, NTILES, TS * 128), np.int16)
    dstloc = np.full((NCORES, NTILES, TS * 128), -1.0, np.float32)

    key = (dst_core.astype(np.int64) * NTILES + dst_tile) * 2 + eh
    es = np.argsort(key, kind="stable")
    ksrc = srcrow[es]; kdl = dst_lane[es]; kds = dstslot[es]
    kc = dst_core[es]; kt = dst_tile[es]; kh = eh[es]
    gkey = key[es]
    start = np.zeros(len(es), bool)
    start[0] = True
    start[1:] = gkey[1:] != gkey[:-1]
    gs = np.where(start, np.arange(len(es)), 0)
    gidx = np.arange(len(es)) - np.maximum.accumulate(gs)
    off = np.where(kh == 0, 0, TA * 128) + gidx
    idxXL[kc, kt, off] = np.where(kh == 0, ksrc, ksrc - HALFR).astype(np.int16)
    idxXR[kc, kt, off] = (kds % SPC).astype(np.int16)
    dstloc[kc, kt, off] = kdl.astype(np.float32)

    dstloc_self = np.full((NCORES, NTILES, LANES), -1.0, np.float32)
    vsel = np.where(self_eids >= 0)[0]
    dstloc_self[core_of_node[vsel], tile_of_node[vsel],
                lane_of_node[vsel]] = lane_of_node[vsel].astype(np.float32)

    return dict(slot_of_node=slot_of_node, node_of_slot=node_of_slot,
                idxXL=idxXL, idxXR=idxXR, dstloc=dstloc,
                dstloc_self=dstloc_self)


def _wrap_idx(idx):
    """[n] -> [128, n//16] wrapped (j at partition j%16, col j//16) + replicated."""
    n = idx.shape[0]
    a = idx.reshape(n // 16, 16).T.astype(np.int16)
    return np.tile(a, (8, 1))


# ---------------------------------------------------------------- device kernel
def _build_nc():
    import concourse.bass as bass
    import concourse.bacc as bacc
    import concourse.tile as tile
    import concourse.mybir as mybir

    F32 = mybir.dt.float32
    BF16 = mybir.dt.bfloat16
    I16 = mybir.dt.int16
    AF = mybir.ActivationFunctionType
    OP = mybir.AluOpType

    LR1, LR2 = _LR_RANGES["l1"], _LR_RANGES["l2"]
    nc = bacc.Bacc(None, target_bir_lowering=False, num_swdge_queues=4)

    # ---- inputs
    xoT = nc.dram_tensor("xoT", [128, SPC], BF16, kind="ExternalInput")
    wl1 = nc.dram_tensor("wl1", [128, HC], BF16, kind="ExternalInput")
    wr1 = nc.dram_tensor("wr1", [128, HC], BF16, kind="ExternalInput")
    wl2 = nc.dram_tensor("wl2", [HC, HC], BF16, kind="ExternalInput")
    wr2 = nc.dram_tensor("wr2", [HC, HC], BF16, kind="ExternalInput")
    w3 = nc.dram_tensor("w3", [HC, 128], BF16, kind="ExternalInput")
    w4 = nc.dram_tensor("w4", [128, OUT_F], BF16, kind="ExternalInput")
    iavT1 = nc.dram_tensor("iavT1", [128, 2], F32, kind="ExternalInput")
    iavT2 = nc.dram_tensor("iavT2", [128, 2], F32, kind="ExternalInput")
    bT1 = nc.dram_tensor("bT1", [128, 2], F32, kind="ExternalInput")
    bT2 = nc.dram_tensor("bT2", [128, 2], F32, kind="ExternalInput")
    b3c = nc.dram_tensor("b3c", [128, 1], F32, kind="ExternalInput")
    b4f = nc.dram_tensor("b4f", [128, OUT_F], F32, kind="ExternalInput")
    iotaBF = nc.dram_tensor("iotaBF", [128, 128], BF16, kind="ExternalInput")
    idenBF = nc.dram_tensor("idenBF", [128, 128], BF16, kind="ExternalInput")
    iotaP = nc.dram_tensor("iotaP", [128, 1], F32, kind="ExternalInput")
    epsc = nc.dram_tensor("epsc", [128, 1], F32, kind="ExternalInput")
    idxXLA = nc.dram_tensor("idxXLA", [NTILES, 128, TA * 8], I16,
                            kind="ExternalInput")
    idxXLB = nc.dram_tensor("idxXLB", [NTILES, 128, TB * 8], I16,
                            kind="ExternalInput")
    idxXR7 = nc.dram_tensor("idxXR7", [NTILES, 128, XG * 8], I16,
                            kind="ExternalInput")
    dstloc = nc.dram_tensor("dstloc", [NTILES, 128, TS + 1], BF16,
                            kind="ExternalInput")
    dlrow = nc.dram_tensor("dlrow", [NTILES, XP * 128], BF16,
                           kind="ExternalInput")
    out_ext = nc.dram_tensor("out", [SPC, OUT_F], F32, kind="ExternalOutput")

    # ---- DRAM intermediates (a/b = AllGather chunk split at tile 24)
    RA = AG_CH[0] * 128           # own rows in chunk a (3072)
    RB = AG_CH[1] * 128           # own rows in chunk b (3200)
    loc1 = nc.dram_tensor("loc1", [SPC, 2, HC], BF16)
    loc2 = nc.dram_tensor("loc2", [SPC, 2, HC], BF16)
    xl_own1a = nc.dram_tensor("xl_own1a", [RA, HC], BF16)
    xl_own1b = nc.dram_tensor("xl_own1b", [RB, HC], BF16)
    xl_own2a = nc.dram_tensor("xl_own2a", [RA, HC], BF16)
    xl_own2b = nc.dram_tensor("xl_own2b", [RB, HC], BF16)
    xl_all1a = nc.dram_tensor("xl_all1a", [HALFR, HC], BF16,
                              addr_space="Shared")
    xl_all1b = nc.dram_tensor("xl_all1b", [S - HALFR, HC], BF16,
                              addr_space="Shared")
    xl_all2a = nc.dram_tensor("xl_all2a", [HALFR, HC], BF16,
                              addr_space="Shared")
    xl_all2b = nc.dram_tensor("xl_all2b", [S - HALFR, HC], BF16,
                              addr_space="Shared")

    with tile.TileContext(nc) as tc:
        with (
            tc.tile_pool(name="const", bufs=1) as cpool,
            tc.tile_pool(name="tabw", bufs=3) as tabw,
            tc.tile_pool(name="gath", bufs=2) as gpool,
            tc.tile_pool(name="work", bufs=2) as wpool,
            tc.tile_pool(name="fin", bufs=2) as fpool,
            tc.tile_pool(name="tps", bufs=2, space="PSUM") as tps,
            tc.tile_pool(name="psu", bufs=2, space="PSUM") as psu,
            tc.tile_pool(name="psx", bufs=1, space="PSUM") as psx,
            tc.tile_pool(name="psd", bufs=1, space="PSUM") as psd,
            tc.tile_pool(name="psT", bufs=1, space="PSUM") as psT,
        ):
            # ---------- persistent constants in SBUF
            def load_const(t, shape, dt):
                tl = cpool.tile(shape, dt, tag=t.name, name=t.name + "_sb")
                nc.sync.dma_start(out=tl[:], in_=t[:])
                return tl

            wl1_sb = load_const(wl1, [128, HC], BF16)
            wr1_sb = load_const(wr1, [128, HC], BF16)
            w4_sb = load_const(w4, [128, OUT_F], BF16)
            iavT1_sb = load_const(iavT1, [128, 2], F32)
            iavT2_sb = load_const(iavT2, [128, 2], F32)
            bT1_sb = load_const(bT1, [128, 2], F32)
            bT2_sb = load_const(bT2, [128, 2], F32)
            b3c_sb = load_const(b3c, [128, 1], F32)
            b4f_sb = load_const(b4f, [128, OUT_F], F32)
            iota_sb = load_const(iotaBF, [128, 128], BF16)
            iden_sb = load_const(idenBF, [128, 128], BF16)
            iotaP_sb = load_const(iotaP, [128, 1], F32)
            epsc_sb = load_const(epsc, [128, 1], F32)

            ones1 = cpool.tile([1, 128], BF16, tag="ones1")
            nc.vector.memset(ones1[:], 1.0)

            def load_const2(t, cols, tag):
                tl = cpool.tile([128, 2, cols], BF16, tag=tag, name=tag + "_sb")
                nc.sync.dma_start(
                    out=tl[:], in_=t.rearrange("(a p) c -> p a c", p=128))
                return tl

            wl2_sb = load_const2(wl2, HC, "wl2x")
            wr2_sb = load_const2(wr2, HC, "wr2x")
            w3_sb = load_const2(w3, 128, "w3x")

            def own_slice(owna, ownb, t):
                if t < AG_CH[0]:
                    return owna[t * 128:(t + 1) * 128, :]
                tb = t - AG_CH[0]
                return ownb[tb * 128:(tb + 1) * 128, :]

            # ---------- L1 local tables: loc1 + xl_own1
            def table_local_l1():
                for t in range(NTILES):
                    lt = tabw.tile([128, 128], BF16, tag="tablhs")
                    nc.sync.dma_start(out=lt[:],
                                      in_=xoT[:, t * 128:(t + 1) * 128])
                    ot = tabw.tile([128, 2, HC], BF16, tag="tabout")
                    for j, w_sb in ((0, wl1_sb), (1, wr1_sb)):
                        pst = tps.tile([128, HC], F32, tag="tabps")
                        nc.tensor.matmul(pst[:], lt[:], w_sb[:], start=True,
                                         stop=True)
                        if j == 0:
                            nc.vector.tensor_copy(ot[:, j, :], pst[:])
                        else:
                            nc.scalar.activation(ot[:, j, :], pst[:], AF.Copy)
                    nc.scalar.dma_start(
                        out=loc1[t * 128:(t + 1) * 128, :, :], in_=ot[:])
                    nc.sync.dma_start(
                        out=own_slice(xl_own1a, xl_own1b, t), in_=ot[:, 0, :])

            # ---------- edge phase (one conv layer)
            # y_act: number of subtiles whose head-1 y-mult runs on ACT
            # (the rest run on DVE) -- balances the two engines per layer
            def conv_layer(xl_ta, xl_tb, loc_tab, lr_ranges, fin_cb, y_act):
                n_batches = NTILES // GB + (1 if NTILES % GB else 0)
                for bi in range(n_batches):
                    t0 = bi * GB
                    tiles = list(range(t0, min(t0 + GB, NTILES)))
                    nb = len(tiles)
                    ixa = gpool.tile([128, nb, TA * 8], I16, tag="ixa")
                    nc.sync.dma_start(
                        out=ixa[:],
                        in_=idxXLA[t0:t0 + nb].rearrange("t p c -> p t c"))
                    ixb = gpool.tile([128, nb, TB * 8], I16, tag="ixb")
                    nc.sync.dma_start(
                        out=ixb[:],
                        in_=idxXLB[t0:t0 + nb].rearrange("t p c -> p t c"))
                    ixr = gpool.tile([128, nb, XG * 8], I16, tag="ixr")
                    nc.sync.dma_start(
                        out=ixr[:],
                        in_=idxXR7[t0:t0 + nb].rearrange("t p c -> p t c"))
                    dlt = gpool.tile([1, nb, XP * 128], BF16, tag="dlt")
                    nc.sync.dma_start(
                        out=dlt[:],
                        in_=dlrow[t0:t0 + nb].rearrange("t c -> () t c"))
                    dli = gpool.tile([128, nb, TS + 1], BF16, tag="dli")
                    nc.sync.dma_start(
                        out=dli[:],
                        in_=dstloc[t0:t0 + nb].rearrange("t p c -> p t c"))
                    sxb = gpool.tile([128, nb, 2, HC], BF16, tag="sxb")
                    nc.sync.dma_start(
                        out=sxb[:],
                        in_=loc_tab[t0 * 128:(t0 + nb) * 128].rearrange(
                            "(a p) b c -> p a b c", p=128))

                    # gathers (triple-buffered so drain hides under compute)
                    gA = gpool.tile([128, nb * TA, HC], BF16, tag="gA", bufs=3)
                    gB = gpool.tile([128, nb * TB, HC], BF16, tag="gB", bufs=3)
                    gR = gpool.tile([128, nb * XG, HC], BF16, tag="gR", bufs=3)
                    nsa = nb * TA
                    ixa_f = ixa[:].rearrange("p t c -> p (t c)")
                    ixb_f = ixb[:].rearrange("p t c -> p (t c)")
                    ixr_f = ixr[:].rearrange("p t c -> p (t c)")
                    nc.gpsimd.dma_gather(
                        out_ap=gA[:], in_ap=xl_ta[:, :],
                        idxs_ap=ixa_f[:],
                        num_idxs=nsa * 128, num_idxs_reg=nsa * 128,
                        elem_size=HC, single_packet=False, queue_num=0)
                    nc.gpsimd.dma_gather(
                        out_ap=gB[:], in_ap=xl_tb[:, :],
                        idxs_ap=ixb_f[:],
                        num_idxs=nsa * 128, num_idxs_reg=nsa * 128,
                        elem_size=HC, single_packet=False, queue_num=1)
                    nsr = nb * XG
                    h2 = nsr // 2
                    nc.gpsimd.dma_gather(
                        out_ap=gR[:, 0:h2, :], in_ap=loc_tab[:, 1, :],
                        idxs_ap=ixr_f[:, 0:h2 * 8],
                        num_idxs=h2 * 128, num_idxs_reg=h2 * 128,
                        elem_size=HC, elem_step=2 * HC, single_packet=False,
                        queue_num=2)
                    nc.gpsimd.dma_gather(
                        out_ap=gR[:, h2:nsr, :], in_ap=loc_tab[:, 1, :],
                        idxs_ap=ixr_f[:, h2 * 8:nsr * 8],
                        num_idxs=(nsr - h2) * 128, num_idxs_reg=(nsr - h2) * 128,
                        elem_size=HC, elem_step=2 * HC, single_packet=False,
                        queue_num=3)

                    NS = TS + 1   # subtiles per tile incl self
                    # 258 cols: [256 values][2 alpha] per subtile row
                    work = wpool.tile([128, nb * NS, 258], BF16, tag="work")
                    w4d = work[:, :, 0:HC].rearrange(
                        "p (t s) c -> p t s c", s=NS)
                    alpha = work[:, :, HC:HC + 2]
                    mk = wpool.tile([128, nb, NS, 128], BF16, tag="mk")
                    mkT = wpool.tile([128, nb, XP, 128], BF16, tag="mkT")

                    # addA: subtiles 0..XG-1 = gA + gathered xr
                    nc.vector.tensor_tensor(
                        out=w4d[:, :, 0:XG, :],
                        in0=gA[:].rearrange("p (t s) c -> p t s c", s=TA),
                        in1=gR[:].rearrange("p (t s) c -> p t s c", s=XG),
                        op=OP.add)
                    # self subtile: loc xl + xr
                    nc.vector.tensor_tensor(
                        out=w4d[:, :, TS, :],
                        in0=sxb[:, :, 0, :], in1=sxb[:, :, 1, :], op=OP.add)

                    # per-tile: mkT build + xr broadcast + addB (2 chunks to
                    # fit fp32 PSUM in banks)
                    for ti in range(nb):
                        for (s0, s1) in ((0, 4), (4, XP)):
                            ns_c = s1 - s0
                            dlb = psd.tile([128, 4 * 128], F32, tag="dlb")
                            nc.tensor.matmul(
                                dlb[:, 0:ns_c * 128], ones1[:],
                                dlt[:, ti, s0 * 128:s1 * 128],
                                start=True, stop=True)
                            nc.vector.tensor_scalar(
                                out=mkT[:, ti, s0:s1, :],
                                in0=dlb[:, 0:ns_c * 128].rearrange(
                                    "p (s c) -> p s c", c=128),
                                scalar1=iotaP_sb[:, 0:1], scalar2=None,
                                op0=OP.is_equal)
                            xrb = psx.tile([128, 4, HC], F32, tag="xrb")
                            for si in range(s0, s1):
                                nc.tensor.matmul(
                                    xrb[:, si - s0, :], mkT[:, ti, si, :],
                                    sxb[:, ti, 1, :], start=True, stop=True)
                            nc.vector.tensor_tensor(
                                out=w4d[:, ti, XG + s0:XG + s1, :],
                                in0=gB[:, ti * TB + s0:ti * TB + s1, :],
                                in1=xrb[:, 0:ns_c, :], op=OP.add)

                    # leaky relu in place (tables pre-scaled by att: max on
                    # +att cols, min on -att cols)
                    for (c0, c1, mop) in lr_ranges:
                        nc.vector.scalar_tensor_tensor(
                            out=work[:, :, c0:c1], in0=work[:, :, c0:c1],
                            scalar=SLOPE, in1=work[:, :, c0:c1],
                            op0=OP.mult,
                            op1=OP.max if mop == "max" else OP.min)

                    # scores: fold 128 -> 32 with cheap adds, then reduce
                    wh = work[:, :, 0:HC].rearrange("p s (h c) -> p s h c", h=2)
                    nc.vector.tensor_tensor(
                        out=wh[:, :, :, 0:64], in0=wh[:, :, :, 0:64],
                        in1=wh[:, :, :, 64:128], op=OP.add)
                    nc.vector.tensor_tensor(
                        out=wh[:, :, :, 0:32], in0=wh[:, :, :, 0:32],
                        in1=wh[:, :, :, 32:64], op=OP.add)
                    sc = wpool.tile([128, nb * NS, 2], F32, tag="sc")
                    nc.vector.tensor_reduce(
                        out=sc[:].rearrange("p s h -> p s h ()"),
                        in_=wh[:, :, :, 0:32],
                        axis=mybir.AxisListType.X, op=OP.add)
                    af = wpool.tile([128, nb * NS, 2], F32, tag="af")
                    nc.scalar.activation(af[:], sc[:], AF.Exp)
                    nc.scalar.activation(alpha, af[:], AF.Copy)

                    # masks for scatter: mk[e, (ti s), d] = (dl==d)
                    nc.vector.tensor_tensor(
                        out=mk[:].rearrange("p t s c -> p (t s) c"),
                        in0=dli[:].rearrange("p t s -> p (t s) ()").broadcast_to(
                            [128, nb * NS, 128]),
                        in1=iota_sb[:].rearrange("p c -> p () c").broadcast_to(
                            [128, nb * NS, 128]),
                        op=OP.is_equal)

                    # y = alpha * xl (overwrites u in work); head 0 on DVE,
                    # head 1 on ACT via per-partition alpha scale
                    for ti in range(nb):
                        ab0 = alpha[:, ti * NS:(ti + 1) * NS, 0:1].broadcast_to(
                            [128, NS, 128])
                        afr = af[:, ti * NS:(ti + 1) * NS, :]
                        nc.vector.tensor_tensor(
                            out=w4d[:, ti, 0:TA, 0:128],
                            in0=gA[:, ti * TA:(ti + 1) * TA, 0:128],
                            in1=ab0[:, 0:TA], op=OP.mult)
                        nc.vector.tensor_tensor(
                            out=w4d[:, ti, TA:TS, 0:128],
                            in0=gB[:, ti * TB:(ti + 1) * TB, 0:128],
                            in1=ab0[:, TA:TS], op=OP.mult)
                        nc.vector.tensor_tensor(
                            out=w4d[:, ti, TS, 0:128],
                            in0=sxb[:, ti, 0, 0:128],
                            in1=ab0[:, TS], op=OP.mult)
                        for s in range(y_act):
                            if s < TA:
                                src_h1 = gA[:, ti * TA + s, 128:HC]
                            elif s < TS:
                                src_h1 = gB[:, ti * TB + (s - TA), 128:HC]
                            else:
                                src_h1 = sxb[:, ti, 0, 128:HC]
                            nc.scalar.activation(
                                w4d[:, ti, s, 128:HC], src_h1, AF.Identity,
                                scale=afr[:, s, 1:2])
                        ab1 = alpha[:, ti * NS:(ti + 1) * NS, 1:2].broadcast_to(
                            [128, NS, 128])
                        if y_act < TS:
                            nc.vector.tensor_tensor(
                                out=w4d[:, ti, y_act:TS, 128:HC],
                                in0=gB[:, ti * TB + (y_act - TA):
                                       (ti + 1) * TB, 128:HC],
                                in1=ab1[:, y_act:TS], op=OP.mult)
                        if y_act < NS:
                            nc.vector.tensor_tensor(
                                out=w4d[:, ti, TS, 128:HC],
                                in0=sxb[:, ti, 0, 128:HC],
                                in1=ab1[:, TS], op=OP.mult)

                    # scatter-accumulate per tile, then finalize
                    for ti, t in enumerate(tiles):
                        u_ps = psu.tile([128, 258], F32, tag="u")
                        for si in range(NS):
                            nc.tensor.matmul(
                                u_ps[:], mk[:, ti, si, :],
                                work[:, ti * NS + si, :],
                                start=(si == 0), stop=(si == NS - 1))
                        fin_cb(t, u_ps)

            # ---------- finalize: u -> h tile (transposed, relu'd)
            def fin_common(u_ps, iavT_sb, bT_sb):
                dcol = fpool.tile([128, 2], F32, tag="dcol")
                nc.scalar.activation(dcol[:], u_ps[:, HC:HC + 2],
                                     AF.Identity, bias=epsc_sb[:, 0:1])
                rcol = fpool.tile([128, 2], F32, tag="rcol")
                nc.vector.reciprocal(rcol[:], dcol[:])
                t1 = fpool.tile([128, 2, 128], BF16, tag="t1")
                for h in range(2):
                    nc.scalar.activation(t1[:, h, :],
                                         u_ps[:, h * 128:(h + 1) * 128],
                                         AF.Identity, scale=rcol[:, h:h + 1])
                cts = []
                for h in range(2):
                    pt = psT.tile([128, 128], BF16, tag="fps")
                    nc.tensor.transpose(pt[:], t1[:, h, :], iden_sb[:])
                    ct = fpool.tile([128, 128], BF16, tag=f"ct{h}")
                    nc.scalar.activation(ct[:], pt[:], AF.Relu,
                                         scale=iavT_sb[:, h:h + 1],
                                         bias=bT_sb[:, h:h + 1])
                    cts.append(ct)
                return cts

            def fin1(t, u_ps):
                cts = fin_common(u_ps, iavT1_sb, bT1_sb)
                ot2 = fpool.tile([128, 2, HC], BF16, tag="ot2")
                for j, w2_sb in ((0, wl2_sb), (1, wr2_sb)):
                    pst = tps.tile([128, HC], F32, tag="tabps")
                    nc.tensor.matmul(pst[:], cts[0][:], w2_sb[:, 0, :],
                                     start=True, stop=False)
                    nc.tensor.matmul(pst[:], cts[1][:], w2_sb[:, 1, :],
                                     start=False, stop=True)
                    nc.scalar.activation(ot2[:, j, :], pst[:], AF.Copy)
                nc.scalar.dma_start(
                    out=loc2[t * 128:(t + 1) * 128, :, :], in_=ot2[:])
                nc.sync.dma_start(
                    out=own_slice(xl_own2a, xl_own2b, t), in_=ot2[:, 0, :])

            def fin2(t, u_ps):
                cts = fin_common(u_ps, iavT2_sb, bT2_sb)
                zt_ps = tps.tile([128, HC], F32, tag="tabps")
                nc.tensor.matmul(zt_ps[:, 0:128], w3_sb[:, 0, :], cts[0][:],
                                 start=True, stop=False)
                nc.tensor.matmul(zt_ps[:, 0:128], w3_sb[:, 1, :], cts[1][:],
                                 start=False, stop=True)
                zt_sb = fpool.tile([128, 128], BF16, tag="ztsb")
                nc.scalar.activation(zt_sb[:], zt_ps[:, 0:128], AF.Identity,
                                     bias=b3c_sb[:, 0:1])
                o_ps = tps.tile([128, HC], F32, tag="tabps")
                nc.tensor.matmul(o_ps[:, 0:OUT_F], zt_sb[:], w4_sb[:],
                                 start=True, stop=True)
                o_pre = fpool.tile([128, OUT_F], F32, tag="opre")
                nc.vector.scalar_tensor_tensor(
                    out=o_pre[:], in0=o_ps[:, 0:OUT_F], scalar=1.0,
                    in1=b4f_sb[:], op0=OP.mult, op1=OP.add)
                o_sb = fpool.tile([128, OUT_F], F32, tag="osb")
                nc.scalar.activation(o_sb[:], o_pre[:], AF.Sigmoid)
                nc.sync.dma_start(out=out_ext[t * 128:(t + 1) * 128, :],
                                  in_=o_sb[:])

            # ================= phase schedule =================
            import os as _os
            _upto = int(_os.environ.get("KPHASES", "9"))

            def allgather_chunks(owna, ownb, alla, allb):
                nc.gpsimd.collective_compute(
                    "AllGather", mybir.AluOpType.bypass,
                    replica_groups=[list(range(NCORES))],
                    ins=[owna.ap().opt()], outs=[alla.ap().opt()])
                nc.gpsimd.collective_compute(
                    "AllGather", mybir.AluOpType.bypass,
                    replica_groups=[list(range(NCORES))],
                    ins=[ownb.ap().opt()], outs=[allb.ap().opt()])

            table_local_l1()
            allgather_chunks(xl_own1a, xl_own1b, xl_all1a, xl_all1b)
            if _upto >= 2:
                conv_layer(xl_all1a, xl_all1b, loc1, LR1, fin1, y_act=TA)
            if _upto >= 3:
                allgather_chunks(xl_own2a, xl_own2b, xl_all2a, xl_all2b)
            if _upto >= 4:
                conv_layer(xl_all2a, xl_all2b, loc2, LR2, fin2, y_act=TS + 1)
            else:
                zt = fpool.tile([128, OUT_F], F32, tag="osb")
                nc.vector.memset(zt[:], 0.0)
                for t in range(NTILES):
                    nc.sync.dma_start(out=out_ext[t * 128:(t + 1) * 128, :],
                                      in_=zt[:])

    nc.compile()
    return nc


# ---------------------------------------------------------------- entry point
def kernel(**inputs):
    from concourse import bass_utils

    src = np.asarray(inputs["edge_index"][0], np.int64)
    dst = np.asarray(inputs["edge_index"][1], np.int64)
    x = np.asarray(inputs["x"], np.float32)

    pack = _pack_graph(src, dst)
    nos = pack["node_of_slot"]
    valid = nos >= 0
    x_slot = np.zeros((S, IN_F), np.float32)
    x_slot[valid] = x[nos[valid]]

    def bf(a):
        return np.ascontiguousarray(np.asarray(a, np.float32)).astype(BF)

    # --- per-head column permutation (+att cols first) + pre-scale by att
    def prep_layer(att):
        att = np.asarray(att, np.float32).reshape(2, 128)
        perm = np.zeros(HC, np.int64)
        ranges = []
        for h in range(2):
            a = att[h]
            pos = np.where(a > 0)[0]
            neg = np.where(a <= 0)[0]
            perm[h * 128:(h + 1) * 128] = h * 128 + np.concatenate([pos, neg])
            p = len(pos)
            if p:
                ranges.append((h * 128, h * 128 + p, "max"))
            if p < 128:
                ranges.append((h * 128 + p, (h + 1) * 128, "min"))
        att_p = att.reshape(HC)[perm]
        att_p = np.where(np.abs(att_p) < 1e-30, 1e-30, att_p)
        return perm, att_p, ranges

    perm1, att1p, LR1 = prep_layer(inputs["att1"])
    perm2, att2p, LR2 = prep_layer(inputs["att2"])
    _LR_RANGES["l1"] = LR1
    _LR_RANGES["l2"] = LR2

    Wl1p = np.asarray(inputs["Wl1"], np.float32)[:, perm1] * att1p[None, :]
    Wr1p = np.asarray(inputs["Wr1"], np.float32)[:, perm1] * att1p[None, :]
    Wl2p = (np.asarray(inputs["Wl2"], np.float32)[perm1][:, perm2]
            * att2p[None, :])
    Wr2p = (np.asarray(inputs["Wr2"], np.float32)[perm1][:, perm2]
            * att2p[None, :])
    W3p = np.asarray(inputs["W3"], np.float32)[perm2]
    b1p = np.asarray(inputs["b1"], np.float32)[perm1]
    b2p = np.asarray(inputs["b2"], np.float32)[perm2]

    common = {
        "wl1": bf(Wl1p), "wr1": bf(Wr1p),
        "wl2": bf(Wl2p), "wr2": bf(Wr2p),
        "w3": bf(W3p), "w4": bf(inputs["W4"]),
        "iavT1": np.ascontiguousarray(
            (1.0 / att1p).reshape(2, 128).T.astype(np.float32)),
        "iavT2": np.ascontiguousarray(
            (1.0 / att2p).reshape(2, 128).T.astype(np.float32)),
        "bT1": np.ascontiguousarray(b1p.reshape(2, 128).T.astype(np.float32)),
        "bT2": np.ascontiguousarray(b2p.reshape(2, 128).T.astype(np.float32)),
        "b3c": np.asarray(inputs["b3"], np.float32).reshape(128, 1),
        "b4f": np.tile(np.asarray(inputs["b4"], np.float32)[None, :], (128, 1)),
        "iotaBF": np.tile(np.arange(128, dtype=np.float32), (128, 1)).astype(BF),
        "idenBF": np.eye(128, dtype=np.float32).astype(BF),
        "iotaP": np.arange(128, dtype=np.float32).reshape(128, 1),
        "epsc": np.full((128, 1), 1e-16, np.float32),
    }

    in_maps = []
    for k in range(NCORES):
        m = dict(common)
        m["xoT"] = np.ascontiguousarray(
            x_slot[k * SPC:(k + 1) * SPC].T).astype(BF)
        ixla = np.empty((NTILES, 128, TA * 8), np.int16)
        ixlb = np.empty((NTILES, 128, TB * 8), np.int16)
        ixr7 = np.empty((NTILES, 128, XG * 8), np.int16)
        dlc = np.empty((NTILES, 128, TS + 1), np.float32)
        dlr = np.empty((NTILES, XP * 128), np.float32)
        for t in range(NTILES):
            ixla[t] = _wrap_idx(pack["idxXL"][k, t, :TA * 128])
            ixlb[t] = _wrap_idx(pack["idxXL"][k, t, TA * 128:])
            ixr7[t] = _wrap_idx(pack["idxXR"][k, t, :XG * 128])
            dlc[t, :, :TS] = pack["dstloc"][k, t].reshape(TS, 128).T
            dlc[t, :, TS] = pack["dstloc_self"][k, t]
            dlr[t] = pack["dstloc"][k, t, XG * 128:]
        m["idxXLA"] = ixla
        m["idxXLB"] = ixlb
        m["idxXR7"] = ixr7
        m["dstloc"] = dlc.astype(BF)
        m["dlrow"] = dlr.astype(BF)
        in_maps.append(m)

    if "nc" not in _NC_CACHE:
        _NC_CACHE["nc"] = _build_nc()
    nc = _NC_CACHE["nc"]

    res = bass_utils.run_bass_kernel_spmd(nc, in_maps,
                                          core_ids=list(range(NCORES)),
                                          **_RUN_OPTS)
    _LAST_RESULTS["res"] = res
    out_slots = np.concatenate([res.results[k]["out"] for k in range(NCORES)], 0)
    return out_slots[pack["slot_of_node"]].astype(np.float32)



# revision 3
# speedup vs baseline: 1.0615x; 1.0615x over previous
"""GATv2 (2-layer, 2-head) Trainium2 kernel, 8-core SPMD — v3.

vs v2: scatter/broadcast one-hot masks precomputed on host and streamed
from DRAM (no DVE is_eq builds, no ones-matmul row-broadcast), xr[dst]
for ALL random subtiles via PE one-hot matmul (gR DMA gathers dropped),
AllGathers split into 5 tile-chunks fired as soon as each chunk's
tiles are produced (table phase for L1, fin1 during conv1 for L2) so
the collectives hide under compute.
"""
import sys

sys.path.insert(0, "/opt/trn_rl_repo")

import numpy as np
import ml_dtypes

BF = ml_dtypes.bfloat16

# ---- static layout constants (match reference problem sizes) ----
N = 50000
NCORES = 8
LANES = 128
NTILES = 49
SPC = NTILES * LANES          # 6272 slots per core
S = NCORES * SPC              # 50176 total slots
TA = 7                        # table-A gather subtiles per dst-tile
TB = 7
TS = TA + TB                  # random-edge subtiles (self subtile is extra)
NS = TS + 1                   # subtiles per tile incl self
GB = 3                        # dst-tiles per gather batch
IN_F = 128
HC = 256                      # H*C
OUT_F = 40
SLOPE = 0.2
# AllGather chunking: 5 tile groups (sum = NTILES). Groups 0,1 make up
# gather table A, groups 2,3,4 table B (the int16 A/B index split).
AG_CH = (12, 12, 12, 8, 5)
AG_T0 = (0, 12, 24, 36, 44)
AG_TAB = (0, 0, 1, 1, 1)      # which gather table each chunk lands in
NTILES_A = 24                 # tiles in table A
HALFR = NTILES_A * LANES * NCORES   # 24576 rows in table A
# global gather row base of each chunk
_b = [0, 0, 0, 0, 0]
_acc = [0, 0]
for _c in range(5):
    _b[_c] = (0 if AG_TAB[_c] == 0 else HALFR) + _acc[AG_TAB[_c]]
    _acc[AG_TAB[_c]] += AG_CH[_c] * LANES * NCORES
AG_BASE = tuple(_b)

_NC_CACHE = {}
_RUN_OPTS = {}
_LAST_RESULTS = {}
_LR_RANGES = {}


# ---------------------------------------------------------------- host prep
def _pack_graph(src, dst):
    deg = np.bincount(dst, minlength=N)

    is_self = src == dst
    self_eids = np.full(N, -1, np.int64)
    sids = np.where(is_self)[0]
    self_eids[src[sids]] = sids
    rand_mask = np.ones(len(src), bool)
    rand_mask[self_eids[self_eids >= 0]] = False

    nodes_per_core = (N + NCORES - 1) // NCORES
    order = np.argsort(-deg, kind="stable")
    core_edges = np.zeros(NCORES, np.int64)
    core_nodes = np.zeros(NCORES, np.int64)
    core_of_node = np.full(N, -1, np.int32)
    for v in order:
        k = np.argmin(np.where(core_nodes < nodes_per_core, core_edges, 1 << 60))
        core_of_node[v] = k
        core_edges[k] += deg[v]
        core_nodes[k] += 1

    rsrc, rdst = src[rand_mask], dst[rand_mask]

    # --- chunk-group assignment per core (before tile packing): deal nodes
    # round-robin by out-degree so the gather-table halves stay balanced.
    NG = len(AG_CH)
    odeg = np.bincount(rsrc, minlength=N)
    group_of_node = np.full(N, -1, np.int8)
    gcap = [c * LANES for c in AG_CH]
    for k in range(NCORES):
        vs = np.where(core_of_node == k)[0]
        vs = vs[np.argsort(-odeg[vs], kind="stable")]
        cnt = [0] * NG
        gi = 0
        for v in vs:
            while cnt[gi % NG] >= gcap[gi % NG]:
                gi += 1
            group_of_node[v] = gi % NG
            cnt[gi % NG] += 1
            gi += 1
    eh_node = np.asarray(AG_TAB, np.int8)[group_of_node]

    dA = np.bincount(rdst[eh_node[rsrc] == 0], minlength=N)
    dB = np.bincount(rdst[eh_node[rsrc] == 1], minlength=N)
    capA, capB = TA * LANES, TB * LANES

    tile_of_node = np.full(N, -1, np.int32)
    lane_of_node = np.full(N, -1, np.int32)
    for k in range(NCORES):
        for g in range(NG):
            vs = np.where((core_of_node == k) & (group_of_node == g))[0]
            vs = vs[np.argsort(-(dA[vs] + dB[vs]), kind="stable")]
            nv = len(vs)
            ntg = AG_CH[g]
            tile = np.empty(nv, np.int64)
            for i in range(nv):
                r, c = divmod(i, ntg)
                tile[i] = c if r % 2 == 0 else ntg - 1 - c
            loadA = np.bincount(tile, weights=dA[vs],
                                minlength=ntg).astype(np.int64)
            loadB = np.bincount(tile, weights=dB[vs],
                                minlength=ntg).astype(np.int64)
            it = 0
            while (loadA.max() > capA or loadB.max() > capB) and it < 100000:
                it += 1
                t_bad = int(np.argmax(np.maximum(loadA - capA, loadB - capB)))
                overA = loadA[t_bad] - capA >= loadB[t_bad] - capB
                t_good = int(np.argmin(loadA + loadB))
                in_bad = np.where(tile == t_bad)[0]
                in_good = np.where(tile == t_good)[0]
                d_bad = dA[vs[in_bad]] if overA else dB[vs[in_bad]]
                ib = in_bad[np.argmax(d_bad)]
                ig = in_good[np.argmin(dA[vs[in_good]] + dB[vs[in_good]])]
                for i, frm, to in ((ib, t_bad, t_good), (ig, t_good, t_bad)):
                    v = vs[i]
                    tile[i] = to
                    loadA[frm] -= dA[v]; loadA[to] += dA[v]
                    loadB[frm] -= dB[v]; loadB[to] += dB[v]
            if loadA.max() > capA or loadB.max() > capB:
                raise RuntimeError("edge packing failed; need bigger TA/TB")
            tile_of_node[vs] = AG_T0[g] + tile
            for t in range(ntg):
                nodes_t = vs[tile == t]
                lane_of_node[nodes_t] = np.arange(len(nodes_t))

    slot_of_node = (core_of_node.astype(np.int64) * SPC
                    + tile_of_node * LANES + lane_of_node)
    node_of_slot = np.full(S, -1, np.int64)
    node_of_slot[slot_of_node] = np.arange(N)

    # chunk-major gather-table row of each node
    g_arr = group_of_node.astype(np.int64)
    base = np.asarray(AG_BASE, np.int64)[g_arr]
    t0 = np.asarray(AG_T0, np.int64)[g_arr]
    chw = np.asarray(AG_CH, np.int64)[g_arr]
    grow_of_node = (base + core_of_node * chw * LANES
                    + (tile_of_node - t0) * LANES + lane_of_node)

    srcrow = grow_of_node[rsrc]
    dstslot = slot_of_node[rdst]
    dst_core = (dstslot // SPC).astype(np.int32)
    dst_tile = ((dstslot % SPC) // LANES).astype(np.int32)
    dst_lane = (dstslot % LANES).astype(np.int32)
    eh = (srcrow >= HALFR).astype(np.int8)

    idxXL = np.zeros((NCORES, NTILES, TS * 128), np.int16)

    key = (dst_core.astype(np.int64) * NTILES + dst_tile) * 2 + eh
    es = np.argsort(key, kind="stable")
    ksrc = srcrow[es]; kdl = dst_lane[es]
    kc = dst_core[es]; kt = dst_tile[es]; kh = eh[es]
    gkey = key[es]
    start = np.zeros(len(es), bool)
    start[0] = True
    start[1:] = gkey[1:] != gkey[:-1]
    gs = np.where(start, np.arange(len(es)), 0)
    gidx = np.arange(len(es)) - np.maximum.accumulate(gs)
    off = np.where(kh == 0, 0, TA * 128) + gidx
    idxXL[kc, kt, off] = np.where(kh == 0, ksrc, ksrc - HALFR).astype(np.int16)

    # one-hot masks: mk [e-lane -> dst-lane] per subtile (incl self at TS),
    # mkT [dst-lane -> e-lane] per random subtile.
    ksi = (off // 128).astype(np.int64)
    kel = (off % 128).astype(np.int64)
    mk = np.zeros((NCORES, NTILES, 128, NS * 128), np.float32)
    mkT = np.zeros((NCORES, NTILES, 128, TS * 128), np.float32)
    mk[kc, kt, kel, ksi * 128 + kdl] = 1.0
    mkT[kc, kt, kdl, ksi * 128 + kel] = 1.0
    vsel = np.where(self_eids >= 0)[0]
    ln = lane_of_node[vsel].astype(np.int64)
    mk[core_of_node[vsel], tile_of_node[vsel], ln, TS * 128 + ln] = 1.0

    return dict(slot_of_node=slot_of_node, node_of_slot=node_of_slot,
                idxXL=idxXL, mk=mk, mkT=mkT)


def _wrap_idx(idx):
    """[n] -> [128, n//16] wrapped (j at partition j%16, col j//16) + replicated."""
    n = idx.shape[0]
    a = idx.reshape(n // 16, 16).T.astype(np.int16)
    return np.tile(a, (8, 1))


# ---------------------------------------------------------------- device kernel
def _build_nc():
    import concourse.bass as bass
    import concourse.bacc as bacc
    import concourse.tile as tile
    import concourse.mybir as mybir

    F32 = mybir.dt.float32
    BF16 = mybir.dt.bfloat16
    I16 = mybir.dt.int16
    AF = mybir.ActivationFunctionType
    OP = mybir.AluOpType

    LR1, LR2 = _LR_RANGES["l1"], _LR_RANGES["l2"]
    nc = bacc.Bacc(None, target_bir_lowering=False, num_swdge_queues=4)

    # ---- inputs
    xoT = nc.dram_tensor("xoT", [128, SPC], BF16, kind="ExternalInput")
    wl1 = nc.dram_tensor("wl1", [128, HC], BF16, kind="ExternalInput")
    wr1 = nc.dram_tensor("wr1", [128, HC], BF16, kind="ExternalInput")
    wl2 = nc.dram_tensor("wl2", [HC, HC], BF16, kind="ExternalInput")
    wr2 = nc.dram_tensor("wr2", [HC, HC], BF16, kind="ExternalInput")
    w3 = nc.dram_tensor("w3", [HC, 128], BF16, kind="ExternalInput")
    w4 = nc.dram_tensor("w4", [128, OUT_F], BF16, kind="ExternalInput")
    iavT1 = nc.dram_tensor("iavT1", [128, 2], F32, kind="ExternalInput")
    iavT2 = nc.dram_tensor("iavT2", [128, 2], F32, kind="ExternalInput")
    bT1 = nc.dram_tensor("bT1", [128, 2], F32, kind="ExternalInput")
    bT2 = nc.dram_tensor("bT2", [128, 2], F32, kind="ExternalInput")
    b3c = nc.dram_tensor("b3c", [128, 1], F32, kind="ExternalInput")
    b4f = nc.dram_tensor("b4f", [128, OUT_F], F32, kind="ExternalInput")
    idenBF = nc.dram_tensor("idenBF", [128, 128], BF16, kind="ExternalInput")
    epsc = nc.dram_tensor("epsc", [128, 1], F32, kind="ExternalInput")
    idxXLA = nc.dram_tensor("idxXLA", [NTILES, 128, TA * 8], I16,
                            kind="ExternalInput")
    idxXLB = nc.dram_tensor("idxXLB", [NTILES, 128, TB * 8], I16,
                            kind="ExternalInput")
    mkd = nc.dram_tensor("mkd", [NTILES, 128, NS * 128], BF16,
                         kind="ExternalInput")
    mkTd = nc.dram_tensor("mkTd", [NTILES, 128, TS * 128], BF16,
                          kind="ExternalInput")
    out_ext = nc.dram_tensor("out", [SPC, OUT_F], F32, kind="ExternalOutput")

    # ---- DRAM intermediates (a/b = gather table split at tile 24)
    RA = NTILES_A * 128           # own rows in table a (3072)
    RB = (NTILES - NTILES_A) * 128  # own rows in table b (3200)
    loc1 = nc.dram_tensor("loc1", [SPC, 2, HC], BF16)
    loc2 = nc.dram_tensor("loc2", [SPC, 2, HC], BF16)
    xl_own1a = nc.dram_tensor("xl_own1a", [RA, HC], BF16)
    xl_own1b = nc.dram_tensor("xl_own1b", [RB, HC], BF16)
    xl_own2a = nc.dram_tensor("xl_own2a", [RA, HC], BF16)
    xl_own2b = nc.dram_tensor("xl_own2b", [RB, HC], BF16)
    xl_all1a = nc.dram_tensor("xl_all1a", [HALFR, HC], BF16,
                              addr_space="Shared")
    xl_all1b = nc.dram_tensor("xl_all1b", [S - HALFR, HC], BF16,
                              addr_space="Shared")
    xl_all2a = nc.dram_tensor("xl_all2a", [HALFR, HC], BF16,
                              addr_space="Shared")
    xl_all2b = nc.dram_tensor("xl_all2b", [S - HALFR, HC], BF16,
                              addr_space="Shared")

    # per-chunk AllGather metadata: (last_tile, own tensor idx, own_r0,
    # all_r0, nrows_own)
    ag_meta = []
    for c in range(5):
        t0, nt, tab = AG_T0[c], AG_CH[c], AG_TAB[c]
        own_r0 = (t0 - (0 if tab == 0 else NTILES_A)) * 128
        all_r0 = AG_BASE[c] - (0 if tab == 0 else HALFR)
        ag_meta.append((t0 + nt - 1, tab, own_r0, all_r0, nt * 128))

    with tile.TileContext(nc) as tc:
        with (
            tc.tile_pool(name="const", bufs=1) as cpool,
            tc.tile_pool(name="tabw", bufs=3) as tabw,
            tc.tile_pool(name="gath", bufs=2) as gpool,
            tc.tile_pool(name="work", bufs=2) as wpool,
            tc.tile_pool(name="fin", bufs=2) as fpool,
            tc.tile_pool(name="tps", bufs=2, space="PSUM") as tps,
            tc.tile_pool(name="psu", bufs=2, space="PSUM") as psu,
            tc.tile_pool(name="psx", bufs=2, space="PSUM") as psx,
            tc.tile_pool(name="psT", bufs=1, space="PSUM") as psT,
        ):
            # ---------- persistent constants in SBUF
            def load_const(t, shape, dt):
                tl = cpool.tile(shape, dt, tag=t.name, name=t.name + "_sb")
                nc.sync.dma_start(out=tl[:], in_=t[:])
                return tl

            wl1_sb = load_const(wl1, [128, HC], BF16)
            wr1_sb = load_const(wr1, [128, HC], BF16)
            w4_sb = load_const(w4, [128, OUT_F], BF16)
            iavT1_sb = load_const(iavT1, [128, 2], F32)
            iavT2_sb = load_const(iavT2, [128, 2], F32)
            bT1_sb = load_const(bT1, [128, 2], F32)
            bT2_sb = load_const(bT2, [128, 2], F32)
            b3c_sb = load_const(b3c, [128, 1], F32)
            b4f_sb = load_const(b4f, [128, OUT_F], F32)
            iden_sb = load_const(idenBF, [128, 128], BF16)
            epsc_sb = load_const(epsc, [128, 1], F32)

            def load_const2(t, cols, tag):
                tl = cpool.tile([128, 2, cols], BF16, tag=tag, name=tag + "_sb")
                nc.sync.dma_start(
                    out=tl[:], in_=t.rearrange("(a p) c -> p a c", p=128))
                return tl

            wl2_sb = load_const2(wl2, HC, "wl2x")
            wr2_sb = load_const2(wr2, HC, "wr2x")
            w3_sb = load_const2(w3, 128, "w3x")

            def own_slice(owna, ownb, t):
                if t < NTILES_A:
                    return owna[t * 128:(t + 1) * 128, :]
                tb = t - NTILES_A
                return ownb[tb * 128:(tb + 1) * 128, :]

            def ag_fire(owns, alls, t):
                """Fire any AllGather chunk whose last tile is t."""
                for (lt, tab, own_r0, all_r0, nr) in ag_meta:
                    if lt != t:
                        continue
                    nc.gpsimd.collective_compute(
                        "AllGather", mybir.AluOpType.bypass,
                        replica_groups=[list(range(NCORES))],
                        ins=[owns[tab][own_r0:own_r0 + nr, :]],
                        outs=[alls[tab][all_r0:all_r0 + nr * NCORES, :]])

            # ---------- L1 local tables: loc1 + xl_own1 (+ chunked AG)
            def table_local_l1():
                owns = (xl_own1a, xl_own1b)
                alls = (xl_all1a, xl_all1b)
                for t in range(NTILES):
                    lt = tabw.tile([128, 128], BF16, tag="tablhs")
                    nc.sync.dma_start(out=lt[:],
                                      in_=xoT[:, t * 128:(t + 1) * 128])
                    ot = tabw.tile([128, 2, HC], BF16, tag="tabout")
                    for j, w_sb in ((0, wl1_sb), (1, wr1_sb)):
                        pst = tps.tile([128, HC], F32, tag="tabps")
                        nc.tensor.matmul(pst[:], lt[:], w_sb[:], start=True,
                                         stop=True)
                        if j == 0:
                            nc.vector.tensor_copy(ot[:, j, :], pst[:])
                        else:
                            nc.scalar.activation(ot[:, j, :], pst[:], AF.Copy)
                    nc.scalar.dma_start(
                        out=loc1[t * 128:(t + 1) * 128, :, :], in_=ot[:])
                    nc.sync.dma_start(
                        out=own_slice(xl_own1a, xl_own1b, t), in_=ot[:, 0, :])
                    ag_fire(owns, alls, t)

            # ---------- edge phase (one conv layer)
            # y_act: number of subtiles whose head-1 y-mult runs on ACT
            # (the rest run on DVE) -- balances the two engines per layer
            def conv_layer(xl_ta, xl_tb, loc_tab, lr_ranges, fin_cb, y_act,
                           ag_cb=None):
                n_batches = NTILES // GB + (1 if NTILES % GB else 0)
                for bi in range(n_batches):
                    t0 = bi * GB
                    tiles = list(range(t0, min(t0 + GB, NTILES)))
                    nb = len(tiles)
                    ixa = gpool.tile([128, nb, TA * 8], I16, tag="ixa")
                    nc.sync.dma_start(
                        out=ixa[:],
                        in_=idxXLA[t0:t0 + nb].rearrange("t p c -> p t c"))
                    ixb = gpool.tile([128, nb, TB * 8], I16, tag="ixb")
                    nc.sync.dma_start(
                        out=ixb[:],
                        in_=idxXLB[t0:t0 + nb].rearrange("t p c -> p t c"))
                    mk = gpool.tile([128, nb, NS * 128], BF16, tag="mk")
                    nc.sync.dma_start(
                        out=mk[:],
                        in_=mkd[t0:t0 + nb].rearrange("t p c -> p t c"))
                    mkT = gpool.tile([128, nb, TS * 128], BF16, tag="mkT")
                    nc.sync.dma_start(
                        out=mkT[:],
                        in_=mkTd[t0:t0 + nb].rearrange("t p c -> p t c"))
                    sxb = gpool.tile([128, nb, 2, HC], BF16, tag="sxb")
                    nc.sync.dma_start(
                        out=sxb[:],
                        in_=loc_tab[t0 * 128:(t0 + nb) * 128].rearrange(
                            "(a p) b c -> p a b c", p=128))

                    # gathers (triple-buffered so drain hides under compute)
                    gA = gpool.tile([128, nb * TA, HC], BF16, tag="gA", bufs=3)
                    gB = gpool.tile([128, nb * TB, HC], BF16, tag="gB", bufs=3)
                    nsa = nb * TA
                    ixa_f = ixa[:].rearrange("p t c -> p (t c)")
                    ixb_f = ixb[:].rearrange("p t c -> p (t c)")
                    nc.gpsimd.dma_gather(
                        out_ap=gA[:], in_ap=xl_ta[:, :],
                        idxs_ap=ixa_f[:],
                        num_idxs=nsa * 128, num_idxs_reg=nsa * 128,
                        elem_size=HC, single_packet=False, queue_num=0)
                    nc.gpsimd.dma_gather(
                        out_ap=gB[:], in_ap=xl_tb[:, :],
                        idxs_ap=ixb_f[:],
                        num_idxs=nsa * 128, num_idxs_reg=nsa * 128,
                        elem_size=HC, single_packet=False, queue_num=1)

                    # 258 cols: [256 values][2 alpha] per subtile row
                    work = wpool.tile([128, nb * NS, 258], BF16, tag="work")
                    w4d = work[:, :, 0:HC].rearrange(
                        "p (t s) c -> p t s c", s=NS)
                    alpha = work[:, :, HC:HC + 2]

                    # xr broadcast via one-hot PE matmul + DVE add with the
                    # gathered xl.  2-subtile PSUM chunks (1 bank each).
                    for ti in range(nb):
                        for s0 in range(0, TS, 2):
                            s1 = s0 + 2
                            xrb = psx.tile([128, 2, HC], F32, tag="xrb")
                            for si in range(s0, s1):
                                nc.tensor.matmul(
                                    xrb[:, si - s0, :],
                                    mkT[:, ti, si * 128:(si + 1) * 128],
                                    sxb[:, ti, 1, :], start=True, stop=True)
                            if s1 <= TA or s0 >= TA:
                                g, gofs = (gA, 0) if s1 <= TA else (gB, TA)
                                nc.vector.tensor_tensor(
                                    out=w4d[:, ti, s0:s1, :],
                                    in0=g[:, ti * TA + s0 - gofs:
                                          ti * TA + s1 - gofs, :],
                                    in1=xrb[:], op=OP.add)
                            else:   # straddles the gA/gB boundary
                                nc.vector.tensor_tensor(
                                    out=w4d[:, ti, s0:s0 + 1, :],
                                    in0=gA[:, ti * TA + s0:ti * TA + s0 + 1, :],
                                    in1=xrb[:, 0:1, :], op=OP.add)
                                nc.vector.tensor_tensor(
                                    out=w4d[:, ti, TA:TA + 1, :],
                                    in0=gB[:, ti * TB:ti * TB + 1, :],
                                    in1=xrb[:, 1:2, :], op=OP.add)
                    # self subtile: loc xl + xr
                    nc.vector.tensor_tensor(
                        out=w4d[:, :, TS, :],
                        in0=sxb[:, :, 0, :], in1=sxb[:, :, 1, :], op=OP.add)

                    # leaky relu in place (tables pre-scaled by att: max on
                    # +att cols, min on -att cols)
                    for (c0, c1, mop) in lr_ranges:
                        nc.vector.scalar_tensor_tensor(
                            out=work[:, :, c0:c1], in0=work[:, :, c0:c1],
                            scalar=SLOPE, in1=work[:, :, c0:c1],
                            op0=OP.mult,
                            op1=OP.max if mop == "max" else OP.min)

                    # scores: fold 128 -> 32 with cheap adds, then reduce
                    wh = work[:, :, 0:HC].rearrange("p s (h c) -> p s h c", h=2)
                    nc.vector.tensor_tensor(
                        out=wh[:, :, :, 0:64], in0=wh[:, :, :, 0:64],
                        in1=wh[:, :, :, 64:128], op=OP.add)
                    nc.vector.tensor_tensor(
                        out=wh[:, :, :, 0:32], in0=wh[:, :, :, 0:32],
                        in1=wh[:, :, :, 32:64], op=OP.add)
                    sc = wpool.tile([128, nb * NS, 2], F32, tag="sc")
                    nc.vector.tensor_reduce(
                        out=sc[:].rearrange("p s h -> p s h ()"),
                        in_=wh[:, :, :, 0:32],
                        axis=mybir.AxisListType.X, op=OP.add)
                    af = wpool.tile([128, nb * NS, 2], F32, tag="af")
                    nc.scalar.activation(af[:], sc[:], AF.Exp)
                    nc.scalar.activation(alpha, af[:], AF.Copy)

                    # y = alpha * xl (overwrites u in work); head 0 on DVE,
                    # head 1 split ACT/DVE via y_act
                    for ti in range(nb):
                        ab0 = alpha[:, ti * NS:(ti + 1) * NS, 0:1].broadcast_to(
                            [128, NS, 128])
                        afr = af[:, ti * NS:(ti + 1) * NS, :]
                        nc.vector.tensor_tensor(
                            out=w4d[:, ti, 0:TA, 0:128],
                            in0=gA[:, ti * TA:(ti + 1) * TA, 0:128],
                            in1=ab0[:, 0:TA], op=OP.mult)
                        nc.vector.tensor_tensor(
                            out=w4d[:, ti, TA:TS, 0:128],
                            in0=gB[:, ti * TB:(ti + 1) * TB, 0:128],
                            in1=ab0[:, TA:TS], op=OP.mult)
                        nc.vector.tensor_tensor(
                            out=w4d[:, ti, TS, 0:128],
                            in0=sxb[:, ti, 0, 0:128],
                            in1=ab0[:, TS], op=OP.mult)
                        for s in range(y_act):
                            if s < TA:
                                src_h1 = gA[:, ti * TA + s, 128:HC]
                            elif s < TS:
                                src_h1 = gB[:, ti * TB + (s - TA), 128:HC]
                            else:
                                src_h1 = sxb[:, ti, 0, 128:HC]
                            nc.scalar.activation(
                                w4d[:, ti, s, 128:HC], src_h1, AF.Identity,
                                scale=afr[:, s, 1:2])
                        ab1 = alpha[:, ti * NS:(ti + 1) * NS, 1:2].broadcast_to(
                            [128, NS, 128])
                        if y_act < TA:
                            nc.vector.tensor_tensor(
                                out=w4d[:, ti, y_act:TA, 128:HC],
                                in0=gA[:, ti * TA + y_act:(ti + 1) * TA,
                                       128:HC],
                                in1=ab1[:, y_act:TA], op=OP.mult)
                        if y_act < TS:
                            ya = max(y_act, TA)
                            nc.vector.tensor_tensor(
                                out=w4d[:, ti, ya:TS, 128:HC],
                                in0=gB[:, ti * TB + (ya - TA):
                                       (ti + 1) * TB, 128:HC],
                                in1=ab1[:, ya:TS], op=OP.mult)
                        if y_act < NS:
                            nc.vector.tensor_tensor(
                                out=w4d[:, ti, TS, 128:HC],
                                in0=sxb[:, ti, 0, 128:HC],
                                in1=ab1[:, TS], op=OP.mult)

                    # scatter-accumulate per tile, then finalize
                    for ti, t in enumerate(tiles):
                        u_ps = psu.tile([128, 258], F32, tag="u")
                        for si in range(NS):
                            nc.tensor.matmul(
                                u_ps[:], mk[:, ti, si * 128:(si + 1) * 128],
                                work[:, ti * NS + si, :],
                                start=(si == 0), stop=(si == NS - 1))
                        fin_cb(t, u_ps)
                        if ag_cb is not None:
                            ag_cb(t)

            # ---------- finalize: u -> h tile (transposed, relu'd)
            def fin_common(u_ps, iavT_sb, bT_sb):
                dcol = fpool.tile([128, 2], F32, tag="dcol")
                nc.scalar.activation(dcol[:], u_ps[:, HC:HC + 2],
                                     AF.Identity, bias=epsc_sb[:, 0:1])
                rcol = fpool.tile([128, 2], F32, tag="rcol")
                nc.vector.reciprocal(rcol[:], dcol[:])
                t1 = fpool.tile([128, 2, 128], BF16, tag="t1")
                for h in range(2):
                    nc.scalar.activation(t1[:, h, :],
                                         u_ps[:, h * 128:(h + 1) * 128],
                                         AF.Identity, scale=rcol[:, h:h + 1])
                cts = []
                for h in range(2):
                    pt = psT.tile([128, 128], BF16, tag="fps")
                    nc.tensor.transpose(pt[:], t1[:, h, :], iden_sb[:])
                    ct = fpool.tile([128, 128], BF16, tag=f"ct{h}")
                    nc.scalar.activation(ct[:], pt[:], AF.Relu,
                                         scale=iavT_sb[:, h:h + 1],
                                         bias=bT_sb[:, h:h + 1])
                    cts.append(ct)
                return cts

            def fin1(t, u_ps):
                cts = fin_common(u_ps, iavT1_sb, bT1_sb)
                ot2 = fpool.tile([128, 2, HC], BF16, tag="ot2")
                for j, w2_sb in ((0, wl2_sb), (1, wr2_sb)):
                    pst = tps.tile([128, HC], F32, tag="tabps")
                    nc.tensor.matmul(pst[:], cts[0][:], w2_sb[:, 0, :],
                                     start=True, stop=False)
                    nc.tensor.matmul(pst[:], cts[1][:], w2_sb[:, 1, :],
                                     start=False, stop=True)
                    nc.scalar.activation(ot2[:, j, :], pst[:], AF.Copy)
                nc.scalar.dma_start(
                    out=loc2[t * 128:(t + 1) * 128, :, :], in_=ot2[:])
                nc.sync.dma_start(
                    out=own_slice(xl_own2a, xl_own2b, t), in_=ot2[:, 0, :])

            def fin2(t, u_ps):
                cts = fin_common(u_ps, iavT2_sb, bT2_sb)
                zt_ps = tps.tile([128, HC], F32, tag="tabps")
                nc.tensor.matmul(zt_ps[:, 0:128], w3_sb[:, 0, :], cts[0][:],
                                 start=True, stop=False)
                nc.tensor.matmul(zt_ps[:, 0:128], w3_sb[:, 1, :], cts[1][:],
                                 start=False, stop=True)
                zt_sb = fpool.tile([128, 128], BF16, tag="ztsb")
                nc.scalar.activation(zt_sb[:], zt_ps[:, 0:128], AF.Identity,
                                     bias=b3c_sb[:, 0:1])
                o_ps = tps.tile([128, HC], F32, tag="tabps")
                nc.tensor.matmul(o_ps[:, 0:OUT_F], zt_sb[:], w4_sb[:],
                                 start=True, stop=True)
                o_pre = fpool.tile([128, OUT_F], F32, tag="opre")
                nc.vector.scalar_tensor_tensor(
                    out=o_pre[:], in0=o_ps[:, 0:OUT_F], scalar=1.0,
                    in1=b4f_sb[:], op0=OP.mult, op1=OP.add)
                o_sb = fpool.tile([128, OUT_F], F32, tag="osb")
                nc.scalar.activation(o_sb[:], o_pre[:], AF.Sigmoid)
                nc.sync.dma_start(out=out_ext[t * 128:(t + 1) * 128, :],
                                  in_=o_sb[:])

            # ================= phase schedule =================
            import os as _os
            _upto = int(_os.environ.get("KPHASES", "9"))

            table_local_l1()
            if _upto >= 2:
                ag2 = lambda t: ag_fire((xl_own2a, xl_own2b),
                                        (xl_all2a, xl_all2b), t)
                conv_layer(xl_all1a, xl_all1b, loc1, LR1, fin1, y_act=TA,
                           ag_cb=ag2)
            if _upto >= 4:
                conv_layer(xl_all2a, xl_all2b, loc2, LR2, fin2, y_act=TA)
            else:
                zt = fpool.tile([128, OUT_F], F32, tag="osb")
                nc.vector.memset(zt[:], 0.0)
                for t in range(NTILES):
                    nc.sync.dma_start(out=out_ext[t * 128:(t + 1) * 128, :],
                                      in_=zt[:])

    nc.compile()
    return nc


# ---------------------------------------------------------------- entry point
def kernel(**inputs):
    from concourse import bass_utils

    src = np.asarray(inputs["edge_index"][0], np.int64)
    dst = np.asarray(inputs["edge_index"][1], np.int64)
    x = np.asarray(inputs["x"], np.float32)

    pack = _pack_graph(src, dst)
    nos = pack["node_of_slot"]
    valid = nos >= 0
    x_slot = np.zeros((S, IN_F), np.float32)
    x_slot[valid] = x[nos[valid]]

    def bf(a):
        return np.ascontiguousarray(np.asarray(a, np.float32)).astype(BF)

    # --- per-head column permutation (+att cols first) + pre-scale by att
    def prep_layer(att):
        att = np.asarray(att, np.float32).reshape(2, 128)
        perm = np.zeros(HC, np.int64)
        ranges = []
        for h in range(2):
            a = att[h]
            pos = np.where(a > 0)[0]
            neg = np.where(a <= 0)[0]
            perm[h * 128:(h + 1) * 128] = h * 128 + np.concatenate([pos, neg])
            p = len(pos)
            if p:
                ranges.append((h * 128, h * 128 + p, "max"))
            if p < 128:
                ranges.append((h * 128 + p, (h + 1) * 128, "min"))
        att_p = att.reshape(HC)[perm]
        att_p = np.where(np.abs(att_p) < 1e-30, 1e-30, att_p)
        return perm, att_p, ranges

    perm1, att1p, LR1 = prep_layer(inputs["att1"])
    perm2, att2p, LR2 = prep_layer(inputs["att2"])
    _LR_RANGES["l1"] = LR1
    _LR_RANGES["l2"] = LR2

    Wl1p = np.asarray(inputs["Wl1"], np.float32)[:, perm1] * att1p[None, :]
    Wr1p = np.asarray(inputs["Wr1"], np.float32)[:, perm1] * att1p[None, :]
    Wl2p = (np.asarray(inputs["Wl2"], np.float32)[perm1][:, perm2]
            * att2p[None, :])
    Wr2p = (np.asarray(inputs["Wr2"], np.float32)[perm1][:, perm2]
            * att2p[None, :])
    W3p = np.asarray(inputs["W3"], np.float32)[perm2]
    b1p = np.asarray(inputs["b1"], np.float32)[perm1]
    b2p = np.asarray(inputs["b2"], np.float32)[perm2]

    common = {
        "wl1": bf(Wl1p), "wr1": bf(Wr1p),
        "wl2": bf(Wl2p), "wr2": bf(Wr2p),
        "w3": bf(W3p), "w4": bf(inputs["W4"]),
        "iavT1": np.ascontiguousarray(
            (1.0 / att1p).reshape(2, 128).T.astype(np.float32)),
        "iavT2": np.ascontiguousarray(
            (1.0 / att2p).reshape(2, 128).T.astype(np.float32)),
        "bT1": np.ascontiguousarray(b1p.reshape(2, 128).T.astype(np.float32)),
        "bT2": np.ascontiguousarray(b2p.reshape(2, 128).T.astype(np.float32)),
        "b3c": np.asarray(inputs["b3"], np.float32).reshape(128, 1),
        "b4f": np.tile(np.asarray(inputs["b4"], np.float32)[None, :], (128, 1)),
        "idenBF": np.eye(128, dtype=np.float32).astype(BF),
        "epsc": np.full((128, 1), 1e-16, np.float32),
    }

    in_maps = []
    for k in range(NCORES):
        m = dict(common)
        m["xoT"] = np.ascontiguousarray(
            x_slot[k * SPC:(k + 1) * SPC].T).astype(BF)
        ixla = np.empty((NTILES, 128, TA * 8), np.int16)
        ixlb = np.empty((NTILES, 128, TB * 8), np.int16)
        for t in range(NTILES):
            ixla[t] = _wrap_idx(pack["idxXL"][k, t, :TA * 128])
            ixlb[t] = _wrap_idx(pack["idxXL"][k, t, TA * 128:])
        m["idxXLA"] = ixla
        m["idxXLB"] = ixlb
        m["mkd"] = pack["mk"][k].astype(BF)
        m["mkTd"] = pack["mkT"][k].astype(BF)
        in_maps.append(m)

    if "nc" not in _NC_CACHE:
        _NC_CACHE["nc"] = _build_nc()
    nc = _NC_CACHE["nc"]

    res = bass_utils.run_bass_kernel_spmd(nc, in_maps,
                                          core_ids=list(range(NCORES)),
                                          **_RUN_OPTS)
    _LAST_RESULTS["res"] = res
    out_slots = np.concatenate([res.results[k]["out"] for k in range(NCORES)], 0)
    return out_slots[pack["slot_of_node"]].astype(np.float32)


# revision 7
# speedup vs baseline: 1.0795x; 1.0170x over previous
"""GATv2 (2-layer, 2-head) Trainium2 kernel, 8-core SPMD — v3.

vs v2: scatter/broadcast one-hot masks precomputed on host and streamed
from DRAM (no DVE is_eq builds, no ones-matmul row-broadcast), xr[dst]
for ALL random subtiles via PE one-hot matmul (gR DMA gathers dropped),
AllGathers split into 5 tile-chunks fired as soon as each chunk's
tiles are produced (table phase for L1, fin1 during conv1 for L2) so
the collectives hide under compute.
"""
import sys

sys.path.insert(0, "/opt/trn_rl_repo")

import numpy as np
import ml_dtypes

BF = ml_dtypes.bfloat16

# ---- static layout constants (match reference problem sizes) ----
N = 50000
NCORES = 8
LANES = 128
NTILES = 49
SPC = NTILES * LANES          # 6272 slots per core
S = NCORES * SPC              # 50176 total slots
TA = 7                        # table-A gather subtiles per dst-tile
TB = 7
TS = TA + TB                  # random-edge subtiles (self subtile is extra)
NS = TS + 1                   # subtiles per tile incl self
GB = 3                        # dst-tiles per gather batch
IN_F = 128
HC = 256                      # H*C
OUT_F = 40
SLOPE = 0.2
# AllGather chunking: 5 tile groups (sum = NTILES). Groups 0,1 make up
# gather table A, groups 2,3,4 table B (the int16 A/B index split).
AG_CH = (12, 12, 12, 10, 3)
AG_T0 = (0, 12, 24, 36, 46)
AG_TAB = (0, 0, 1, 1, 1)      # which gather table each chunk lands in
NTILES_A = 24                 # tiles in table A
HALFR = NTILES_A * LANES * NCORES   # 24576 rows in table A
# global gather row base of each chunk
_b = [0, 0, 0, 0, 0]
_acc = [0, 0]
for _c in range(5):
    _b[_c] = (0 if AG_TAB[_c] == 0 else HALFR) + _acc[AG_TAB[_c]]
    _acc[AG_TAB[_c]] += AG_CH[_c] * LANES * NCORES
AG_BASE = tuple(_b)

_NC_CACHE = {}
_RUN_OPTS = {}
_LAST_RESULTS = {}
_LR_RANGES = {}


# ---------------------------------------------------------------- host prep
def _pack_graph(src, dst):
    deg = np.bincount(dst, minlength=N)

    is_self = src == dst
    self_eids = np.full(N, -1, np.int64)
    sids = np.where(is_self)[0]
    self_eids[src[sids]] = sids
    rand_mask = np.ones(len(src), bool)
    rand_mask[self_eids[self_eids >= 0]] = False

    nodes_per_core = (N + NCORES - 1) // NCORES
    order = np.argsort(-deg, kind="stable")
    core_edges = np.zeros(NCORES, np.int64)
    core_nodes = np.zeros(NCORES, np.int64)
    core_of_node = np.full(N, -1, np.int32)
    for v in order:
        k = np.argmin(np.where(core_nodes < nodes_per_core, core_edges, 1 << 60))
        core_of_node[v] = k
        core_edges[k] += deg[v]
        core_nodes[k] += 1

    rsrc, rdst = src[rand_mask], dst[rand_mask]

    # --- chunk-group assignment per core (before tile packing): deal nodes
    # round-robin by out-degree so the gather-table halves stay balanced.
    NG = len(AG_CH)
    odeg = np.bincount(rsrc, minlength=N)
    group_of_node = np.full(N, -1, np.int8)
    gcap = [c * LANES for c in AG_CH]
    for k in range(NCORES):
        vs = np.where(core_of_node == k)[0]
        vs = vs[np.argsort(-odeg[vs], kind="stable")]
        cnt = [0] * NG
        gi = 0
        for v in vs:
            while cnt[gi % NG] >= gcap[gi % NG]:
                gi += 1
            group_of_node[v] = gi % NG
            cnt[gi % NG] += 1
            gi += 1
    eh_node = np.asarray(AG_TAB, np.int8)[group_of_node]

    dA = np.bincount(rdst[eh_node[rsrc] == 0], minlength=N)
    dB = np.bincount(rdst[eh_node[rsrc] == 1], minlength=N)
    capA, capB = TA * LANES, TB * LANES

    tile_of_node = np.full(N, -1, np.int32)
    lane_of_node = np.full(N, -1, np.int32)
    for k in range(NCORES):
        for g in range(NG):
            vs = np.where((core_of_node == k) & (group_of_node == g))[0]
            vs = vs[np.argsort(-(dA[vs] + dB[vs]), kind="stable")]
            nv = len(vs)
            ntg = AG_CH[g]
            tile = np.empty(nv, np.int64)
            for i in range(nv):
                r, c = divmod(i, ntg)
                tile[i] = c if r % 2 == 0 else ntg - 1 - c
            loadA = np.bincount(tile, weights=dA[vs],
                                minlength=ntg).astype(np.int64)
            loadB = np.bincount(tile, weights=dB[vs],
                                minlength=ntg).astype(np.int64)
            it = 0
            while (loadA.max() > capA or loadB.max() > capB) and it < 100000:
                it += 1
                t_bad = int(np.argmax(np.maximum(loadA - capA, loadB - capB)))
                overA = loadA[t_bad] - capA >= loadB[t_bad] - capB
                t_good = int(np.argmin(loadA + loadB))
                in_bad = np.where(tile == t_bad)[0]
                in_good = np.where(tile == t_good)[0]
                d_bad = dA[vs[in_bad]] if overA else dB[vs[in_bad]]
                ib = in_bad[np.argmax(d_bad)]
                ig = in_good[np.argmin(dA[vs[in_good]] + dB[vs[in_good]])]
                for i, frm, to in ((ib, t_bad, t_good), (ig, t_good, t_bad)):
                    v = vs[i]
                    tile[i] = to
                    loadA[frm] -= dA[v]; loadA[to] += dA[v]
                    loadB[frm] -= dB[v]; loadB[to] += dB[v]
            if loadA.max() > capA or loadB.max() > capB:
                raise RuntimeError("edge packing failed; need bigger TA/TB")
            tile_of_node[vs] = AG_T0[g] + tile
            for t in range(ntg):
                nodes_t = vs[tile == t]
                lane_of_node[nodes_t] = np.arange(len(nodes_t))

    slot_of_node = (core_of_node.astype(np.int64) * SPC
                    + tile_of_node * LANES + lane_of_node)
    node_of_slot = np.full(S, -1, np.int64)
    node_of_slot[slot_of_node] = np.arange(N)

    # chunk-major gather-table row of each node
    g_arr = group_of_node.astype(np.int64)
    base = np.asarray(AG_BASE, np.int64)[g_arr]
    t0 = np.asarray(AG_T0, np.int64)[g_arr]
    chw = np.asarray(AG_CH, np.int64)[g_arr]
    grow_of_node = (base + core_of_node * chw * LANES
                    + (tile_of_node - t0) * LANES + lane_of_node)

    srcrow = grow_of_node[rsrc]
    dstslot = slot_of_node[rdst]
    dst_core = (dstslot // SPC).astype(np.int32)
    dst_tile = ((dstslot % SPC) // LANES).astype(np.int32)
    dst_lane = (dstslot % LANES).astype(np.int32)
    eh = (srcrow >= HALFR).astype(np.int8)

    idxXL = np.zeros((NCORES, NTILES, TS * 128), np.int16)

    key = (dst_core.astype(np.int64) * NTILES + dst_tile) * 2 + eh
    es = np.argsort(key, kind="stable")
    ksrc = srcrow[es]; kdl = dst_lane[es]
    kc = dst_core[es]; kt = dst_tile[es]; kh = eh[es]
    gkey = key[es]
    start = np.zeros(len(es), bool)
    start[0] = True
    start[1:] = gkey[1:] != gkey[:-1]
    gs = np.where(start, np.arange(len(es)), 0)
    gidx = np.arange(len(es)) - np.maximum.accumulate(gs)
    off = np.where(kh == 0, 0, TA * 128) + gidx
    idxXL[kc, kt, off] = np.where(kh == 0, ksrc, ksrc - HALFR).astype(np.int16)

    # one-hot masks: mk [e-lane -> dst-lane] per subtile (incl self at TS),
    # mkT [dst-lane -> e-lane] per random subtile.
    ksi = (off // 128).astype(np.int64)
    kel = (off % 128).astype(np.int64)
    mk = np.zeros((NCORES, NTILES, 128, NS * 128), np.float32)
    mkT = np.zeros((NCORES, NTILES, 128, TS * 128), np.float32)
    mk[kc, kt, kel, ksi * 128 + kdl] = 1.0
    mkT[kc, kt, kdl, ksi * 128 + kel] = 1.0
    vsel = np.where(self_eids >= 0)[0]
    ln = lane_of_node[vsel].astype(np.int64)
    mk[core_of_node[vsel], tile_of_node[vsel], ln, TS * 128 + ln] = 1.0

    return dict(slot_of_node=slot_of_node, node_of_slot=node_of_slot,
                idxXL=idxXL, mk=mk, mkT=mkT)


def _wrap_idx(idx):
    """[n] -> [128, n//16] wrapped (j at partition j%16, col j//16) + replicated."""
    n = idx.shape[0]
    a = idx.reshape(n // 16, 16).T.astype(np.int16)
    return np.tile(a, (8, 1))


# ---------------------------------------------------------------- device kernel
def _build_nc():
    import concourse.bass as bass
    import concourse.bacc as bacc
    import concourse.tile as tile
    import concourse.mybir as mybir

    F32 = mybir.dt.float32
    BF16 = mybir.dt.bfloat16
    I16 = mybir.dt.int16
    AF = mybir.ActivationFunctionType
    OP = mybir.AluOpType

    LR1, LR2 = _LR_RANGES["l1"], _LR_RANGES["l2"]
    nc = bacc.Bacc(None, target_bir_lowering=False, num_swdge_queues=4)

    # ---- inputs
    xoT = nc.dram_tensor("xoT", [128, SPC], BF16, kind="ExternalInput")
    wl1 = nc.dram_tensor("wl1", [128, HC], BF16, kind="ExternalInput")
    wr1 = nc.dram_tensor("wr1", [128, HC], BF16, kind="ExternalInput")
    wl2 = nc.dram_tensor("wl2", [HC, HC], BF16, kind="ExternalInput")
    wr2 = nc.dram_tensor("wr2", [HC, HC], BF16, kind="ExternalInput")
    w3 = nc.dram_tensor("w3", [HC, 128], BF16, kind="ExternalInput")
    w4 = nc.dram_tensor("w4", [128, OUT_F], BF16, kind="ExternalInput")
    iavT1 = nc.dram_tensor("iavT1", [128, 2], F32, kind="ExternalInput")
    iavT2 = nc.dram_tensor("iavT2", [128, 2], F32, kind="ExternalInput")
    bT1 = nc.dram_tensor("bT1", [128, 2], F32, kind="ExternalInput")
    bT2 = nc.dram_tensor("bT2", [128, 2], F32, kind="ExternalInput")
    b3c = nc.dram_tensor("b3c", [128, 1], F32, kind="ExternalInput")
    b4f = nc.dram_tensor("b4f", [128, OUT_F], F32, kind="ExternalInput")
    idenBF = nc.dram_tensor("idenBF", [128, 128], BF16, kind="ExternalInput")
    epsc = nc.dram_tensor("epsc", [128, 1], F32, kind="ExternalInput")
    idxXLA = nc.dram_tensor("idxXLA", [NTILES, 128, TA * 8], I16,
                            kind="ExternalInput")
    idxXLB = nc.dram_tensor("idxXLB", [NTILES, 128, TB * 8], I16,
                            kind="ExternalInput")
    mkd = nc.dram_tensor("mkd", [NTILES, 128, NS * 128], BF16,
                         kind="ExternalInput")
    mkTd = nc.dram_tensor("mkTd", [NTILES, 128, TS * 128], BF16,
                          kind="ExternalInput")
    out_ext = nc.dram_tensor("out", [SPC, OUT_F], F32, kind="ExternalOutput")

    # ---- DRAM intermediates (a/b = gather table split at tile 24)
    RA = NTILES_A * 128           # own rows in table a (3072)
    RB = (NTILES - NTILES_A) * 128  # own rows in table b (3200)
    loc1 = nc.dram_tensor("loc1", [SPC, 2, HC], BF16)
    loc2 = nc.dram_tensor("loc2", [SPC, 2, HC], BF16)
    xl_own1a = nc.dram_tensor("xl_own1a", [RA, HC], BF16)
    xl_own1b = nc.dram_tensor("xl_own1b", [RB, HC], BF16)
    xl_own2a = nc.dram_tensor("xl_own2a", [RA, HC], BF16)
    xl_own2b = nc.dram_tensor("xl_own2b", [RB, HC], BF16)
    xl_all1a = nc.dram_tensor("xl_all1a", [HALFR, HC], BF16,
                              addr_space="Shared")
    xl_all1b = nc.dram_tensor("xl_all1b", [S - HALFR, HC], BF16,
                              addr_space="Shared")
    xl_all2a = nc.dram_tensor("xl_all2a", [HALFR, HC], BF16,
                              addr_space="Shared")
    xl_all2b = nc.dram_tensor("xl_all2b", [S - HALFR, HC], BF16,
                              addr_space="Shared")

    # per-chunk AllGather metadata: (last_tile, own tensor idx, own_r0,
    # all_r0, nrows_own)
    ag_meta = []
    for c in range(5):
        t0, nt, tab = AG_T0[c], AG_CH[c], AG_TAB[c]
        own_r0 = (t0 - (0 if tab == 0 else NTILES_A)) * 128
        all_r0 = AG_BASE[c] - (0 if tab == 0 else HALFR)
        ag_meta.append((t0 + nt - 1, tab, own_r0, all_r0, nt * 128))

    with tile.TileContext(nc) as tc:
        with (
            tc.tile_pool(name="const", bufs=1) as cpool,
            tc.tile_pool(name="tabw", bufs=3) as tabw,
            tc.tile_pool(name="gath", bufs=2) as gpool,
            tc.tile_pool(name="work", bufs=2) as wpool,
            tc.tile_pool(name="fin", bufs=2) as fpool,
            tc.tile_pool(name="tps", bufs=1, space="PSUM") as tps,
            tc.tile_pool(name="psu", bufs=2, space="PSUM") as psu,
            tc.tile_pool(name="psx", bufs=2, space="PSUM") as psx,
            tc.tile_pool(name="psT", bufs=1, space="PSUM") as psT,
        ):
            # ---------- persistent constants in SBUF
            def load_const(t, shape, dt):
                tl = cpool.tile(shape, dt, tag=t.name, name=t.name + "_sb")
                nc.sync.dma_start(out=tl[:], in_=t[:])
                return tl

            wl1_sb = load_const(wl1, [128, HC], BF16)
            wr1_sb = load_const(wr1, [128, HC], BF16)
            w4_sb = load_const(w4, [128, OUT_F], BF16)
            iavT1_sb = load_const(iavT1, [128, 2], F32)
            iavT2_sb = load_const(iavT2, [128, 2], F32)
            bT1_sb = load_const(bT1, [128, 2], F32)
            bT2_sb = load_const(bT2, [128, 2], F32)
            b3c_sb = load_const(b3c, [128, 1], F32)
            b4f_sb = load_const(b4f, [128, OUT_F], F32)
            iden_sb = load_const(idenBF, [128, 128], BF16)
            epsc_sb = load_const(epsc, [128, 1], F32)

            def load_const2(t, cols, tag):
                tl = cpool.tile([128, 2, cols], BF16, tag=tag, name=tag + "_sb")
                nc.sync.dma_start(
                    out=tl[:], in_=t.rearrange("(a p) c -> p a c", p=128))
                return tl

            wl2_sb = load_const2(wl2, HC, "wl2x")
            wr2_sb = load_const2(wr2, HC, "wr2x")
            w3_sb = load_const2(w3, 128, "w3x")

            def own_slice(owna, ownb, t):
                if t < NTILES_A:
                    return owna[t * 128:(t + 1) * 128, :]
                tb = t - NTILES_A
                return ownb[tb * 128:(tb + 1) * 128, :]

            def ag_fire(owns, alls, t):
                """Fire any AllGather chunk whose last tile is t."""
                for (lt, tab, own_r0, all_r0, nr) in ag_meta:
                    if lt != t:
                        continue
                    nc.gpsimd.collective_compute(
                        "AllGather", mybir.AluOpType.bypass,
                        replica_groups=[list(range(NCORES))],
                        ins=[owns[tab][own_r0:own_r0 + nr, :]],
                        outs=[alls[tab][all_r0:all_r0 + nr * NCORES, :]])

            # ---------- L1 local tables: loc1 + xl_own1 (+ chunked AG)
            def table_local_l1():
                owns = (xl_own1a, xl_own1b)
                alls = (xl_all1a, xl_all1b)
                for t in range(NTILES):
                    lt = tabw.tile([128, 128], BF16, tag="tablhs")
                    nc.sync.dma_start(out=lt[:],
                                      in_=xoT[:, t * 128:(t + 1) * 128])
                    ot = tabw.tile([128, 2, HC], BF16, tag="tabout")
                    for j, w_sb in ((0, wl1_sb), (1, wr1_sb)):
                        pst = tps.tile([128, HC], F32, tag="tabps")
                        nc.tensor.matmul(pst[:], lt[:], w_sb[:], start=True,
                                         stop=True)
                        if j == 0:
                            nc.vector.tensor_copy(ot[:, j, :], pst[:])
                        else:
                            nc.scalar.activation(ot[:, j, :], pst[:], AF.Copy)
                    nc.scalar.dma_start(
                        out=loc1[t * 128:(t + 1) * 128, :, :], in_=ot[:])
                    nc.sync.dma_start(
                        out=own_slice(xl_own1a, xl_own1b, t), in_=ot[:, 0, :])
                    ag_fire(owns, alls, t)

            # ---------- edge phase (one conv layer)
            # y_act: number of subtiles whose head-1 y-mult runs on ACT
            # (the rest run on DVE) -- balances the two engines per layer
            def conv_layer(xl_ta, xl_tb, loc_tab, lr_ranges, fin_cb, y_act,
                           ag_cb=None):
                n_batches = NTILES // GB + (1 if NTILES % GB else 0)
                for bi in range(n_batches):
                    t0 = bi * GB
                    tiles = list(range(t0, min(t0 + GB, NTILES)))
                    nb = len(tiles)
                    ixa = gpool.tile([128, nb, TA * 8], I16, tag="ixa")
                    nc.sync.dma_start(
                        out=ixa[:],
                        in_=idxXLA[t0:t0 + nb].rearrange("t p c -> p t c"))
                    ixb = gpool.tile([128, nb, TB * 8], I16, tag="ixb")
                    nc.sync.dma_start(
                        out=ixb[:],
                        in_=idxXLB[t0:t0 + nb].rearrange("t p c -> p t c"))
                    mk = gpool.tile([128, nb, NS * 128], BF16, tag="mk")
                    nc.sync.dma_start(
                        out=mk[:],
                        in_=mkd[t0:t0 + nb].rearrange("t p c -> p t c"))
                    mkT = gpool.tile([128, nb, TS * 128], BF16, tag="mkT")
                    nc.sync.dma_start(
                        out=mkT[:],
                        in_=mkTd[t0:t0 + nb].rearrange("t p c -> p t c"))
                    sxb = gpool.tile([128, nb, 2, HC], BF16, tag="sxb")
                    nc.sync.dma_start(
                        out=sxb[:],
                        in_=loc_tab[t0 * 128:(t0 + nb) * 128].rearrange(
                            "(a p) b c -> p a b c", p=128))

                    # gathers (triple-buffered so drain hides under compute)
                    gA = gpool.tile([128, nb * TA, HC], BF16, tag="gA", bufs=3)
                    gB = gpool.tile([128, nb * TB, HC], BF16, tag="gB", bufs=3)
                    nsa = nb * TA
                    ixa_f = ixa[:].rearrange("p t c -> p (t c)")
                    ixb_f = ixb[:].rearrange("p t c -> p (t c)")
                    nc.gpsimd.dma_gather(
                        out_ap=gA[:], in_ap=xl_ta[:, :],
                        idxs_ap=ixa_f[:],
                        num_idxs=nsa * 128, num_idxs_reg=nsa * 128,
                        elem_size=HC, single_packet=False, queue_num=0)
                    nc.gpsimd.dma_gather(
                        out_ap=gB[:], in_ap=xl_tb[:, :],
                        idxs_ap=ixb_f[:],
                        num_idxs=nsa * 128, num_idxs_reg=nsa * 128,
                        elem_size=HC, single_packet=False, queue_num=1)

                    # 258 cols: [256 values][2 alpha] per subtile row
                    work = wpool.tile([128, nb * NS, 258], BF16, tag="work")
                    w4d = work[:, :, 0:HC].rearrange(
                        "p (t s) c -> p t s c", s=NS)
                    alpha = work[:, :, HC:HC + 2]

                    # u = xl[src] + xr[dst], both on PE: xr via one-hot
                    # matmul, xl via identity-matmul accumulate into the same
                    # PSUM; ACT drains each 4-subtile chunk into work as bf16.
                    for ti in range(nb):
                        for s0 in range(0, TS, 4):
                            s1 = min(s0 + 4, TS)
                            xrb = psx.tile([128, 4, HC], F32, tag="xrb")
                            for si in range(s0, s1):
                                nc.tensor.matmul(
                                    xrb[:, si - s0, :],
                                    mkT[:, ti, si * 128:(si + 1) * 128],
                                    sxb[:, ti, 1, :], start=True, stop=False)
                                g, gofs = (gA, 0) if si < TA else (gB, TA)
                                nc.tensor.matmul(
                                    xrb[:, si - s0, :], iden_sb[:],
                                    g[:, ti * TA + si - gofs, :],
                                    start=False, stop=True)
                            nc.scalar.activation(
                                w4d[:, ti, s0:s1, :], xrb[:, 0:s1 - s0, :],
                                AF.Copy)
                    # self subtile: loc xl + xr
                    nc.vector.tensor_tensor(
                        out=w4d[:, :, TS, :],
                        in0=sxb[:, :, 0, :], in1=sxb[:, :, 1, :], op=OP.add)

                    # leaky relu in place (tables pre-scaled by att: max on
                    # +att cols, min on -att cols); 2-op form so the
                    # tensor_tensor step can hit the fast DVE modes
                    lk = wpool.tile([128, nb * NS, 128], BF16, tag="lk")
                    for h in range(2):
                        nc.vector.tensor_scalar(
                            out=lk[:], in0=work[:, :, h * 128:(h + 1) * 128],
                            scalar1=SLOPE, scalar2=None, op0=OP.mult)
                        for (c0, c1, mop) in lr_ranges:
                            if c0 >= (h + 1) * 128 or c1 <= h * 128:
                                continue
                            nc.vector.tensor_tensor(
                                out=work[:, :, c0:c1], in0=work[:, :, c0:c1],
                                in1=lk[:, :, c0 - h * 128:c1 - h * 128],
                                op=OP.max if mop == "max" else OP.min)

                    # scores: fold 128 -> 32 with cheap adds, then reduce
                    wh = work[:, :, 0:HC].rearrange("p s (h c) -> p s h c", h=2)
                    nc.vector.tensor_tensor(
                        out=wh[:, :, :, 0:64], in0=wh[:, :, :, 0:64],
                        in1=wh[:, :, :, 64:128], op=OP.add)
                    nc.vector.tensor_tensor(
                        out=wh[:, :, :, 0:32], in0=wh[:, :, :, 0:32],
                        in1=wh[:, :, :, 32:64], op=OP.add)
                    sc = wpool.tile([128, nb * NS, 2], F32, tag="sc")
                    nc.vector.tensor_reduce(
                        out=sc[:].rearrange("p s h -> p s h ()"),
                        in_=wh[:, :, :, 0:32],
                        axis=mybir.AxisListType.X, op=OP.add)
                    af = wpool.tile([128, nb * NS, 2], F32, tag="af")
                    nc.scalar.activation(af[:], sc[:], AF.Exp)
                    nc.scalar.activation(alpha, af[:], AF.Copy)

                    # y = alpha * xl (overwrites u in work); head 0 on DVE,
                    # head 1 split ACT/DVE via y_act
                    for ti in range(nb):
                        ab0 = alpha[:, ti * NS:(ti + 1) * NS, 0:1].broadcast_to(
                            [128, NS, 128])
                        afr = af[:, ti * NS:(ti + 1) * NS, :]
                        nc.vector.tensor_tensor(
                            out=w4d[:, ti, 0:TA, 0:128],
                            in0=gA[:, ti * TA:(ti + 1) * TA, 0:128],
                            in1=ab0[:, 0:TA], op=OP.mult)
                        nc.vector.tensor_tensor(
                            out=w4d[:, ti, TA:TS, 0:128],
                            in0=gB[:, ti * TB:(ti + 1) * TB, 0:128],
                            in1=ab0[:, TA:TS], op=OP.mult)
                        nc.vector.tensor_tensor(
                            out=w4d[:, ti, TS, 0:128],
                            in0=sxb[:, ti, 0, 0:128],
                            in1=ab0[:, TS], op=OP.mult)
                        for s in range(y_act):
                            if s < TA:
                                src_h1 = gA[:, ti * TA + s, 128:HC]
                            elif s < TS:
                                src_h1 = gB[:, ti * TB + (s - TA), 128:HC]
                            else:
                                src_h1 = sxb[:, ti, 0, 128:HC]
                            nc.scalar.activation(
                                w4d[:, ti, s, 128:HC], src_h1, AF.Identity,
                                scale=afr[:, s, 1:2])
                        ab1 = alpha[:, ti * NS:(ti + 1) * NS, 1:2].broadcast_to(
                            [128, NS, 128])
                        if y_act < TA:
                            nc.vector.tensor_tensor(
                                out=w4d[:, ti, y_act:TA, 128:HC],
                                in0=gA[:, ti * TA + y_act:(ti + 1) * TA,
                                       128:HC],
                                in1=ab1[:, y_act:TA], op=OP.mult)
                        if y_act < TS:
                            ya = max(y_act, TA)
                            nc.vector.tensor_tensor(
                                out=w4d[:, ti, ya:TS, 128:HC],
                                in0=gB[:, ti * TB + (ya - TA):
                                       (ti + 1) * TB, 128:HC],
                                in1=ab1[:, ya:TS], op=OP.mult)
                        if y_act < NS:
                            nc.vector.tensor_tensor(
                                out=w4d[:, ti, TS, 128:HC],
                                in0=sxb[:, ti, 0, 128:HC],
                                in1=ab1[:, TS], op=OP.mult)

                    # scatter-accumulate per tile, then finalize
                    for ti, t in enumerate(tiles):
                        u_ps = psu.tile([128, 258], F32, tag="u")
                        for si in range(NS):
                            nc.tensor.matmul(
                                u_ps[:], mk[:, ti, si * 128:(si + 1) * 128],
                                work[:, ti * NS + si, :],
                                start=(si == 0), stop=(si == NS - 1))
                        fin_cb(t, u_ps)
                        if ag_cb is not None:
                            ag_cb(t)

            # ---------- finalize: u -> h tile (transposed, relu'd)
            def fin_common(u_ps, iavT_sb, bT_sb):
                dcol = fpool.tile([128, 2], F32, tag="dcol")
                nc.scalar.activation(dcol[:], u_ps[:, HC:HC + 2],
                                     AF.Identity, bias=epsc_sb[:, 0:1])
                rcol = fpool.tile([128, 2], F32, tag="rcol")
                nc.vector.reciprocal(rcol[:], dcol[:])
                t1 = fpool.tile([128, 2, 128], BF16, tag="t1")
                for h in range(2):
                    nc.scalar.activation(t1[:, h, :],
                                         u_ps[:, h * 128:(h + 1) * 128],
                                         AF.Identity, scale=rcol[:, h:h + 1])
                cts = []
                for h in range(2):
                    pt = psT.tile([128, 128], BF16, tag="fps")
                    nc.tensor.transpose(pt[:], t1[:, h, :], iden_sb[:])
                    ct = fpool.tile([128, 128], BF16, tag=f"ct{h}")
                    nc.scalar.activation(ct[:], pt[:], AF.Relu,
                                         scale=iavT_sb[:, h:h + 1],
                                         bias=bT_sb[:, h:h + 1])
                    cts.append(ct)
                return cts

            def fin1(t, u_ps):
                cts = fin_common(u_ps, iavT1_sb, bT1_sb)
                ot2 = fpool.tile([128, 2, HC], BF16, tag="ot2")
                for j, w2_sb in ((0, wl2_sb), (1, wr2_sb)):
                    pst = tps.tile([128, HC], F32, tag="tabps")
                    nc.tensor.matmul(pst[:], cts[0][:], w2_sb[:, 0, :],
                                     start=True, stop=False)
                    nc.tensor.matmul(pst[:], cts[1][:], w2_sb[:, 1, :],
                                     start=False, stop=True)
                    nc.scalar.activation(ot2[:, j, :], pst[:], AF.Copy)
                nc.scalar.dma_start(
                    out=loc2[t * 128:(t + 1) * 128, :, :], in_=ot2[:])
                nc.sync.dma_start(
                    out=own_slice(xl_own2a, xl_own2b, t), in_=ot2[:, 0, :])

            def fin2(t, u_ps):
                cts = fin_common(u_ps, iavT2_sb, bT2_sb)
                zt_ps = tps.tile([128, HC], F32, tag="tabps")
                nc.tensor.matmul(zt_ps[:, 0:128], w3_sb[:, 0, :], cts[0][:],
                                 start=True, stop=False)
                nc.tensor.matmul(zt_ps[:, 0:128], w3_sb[:, 1, :], cts[1][:],
                                 start=False, stop=True)
                zt_sb = fpool.tile([128, 128], BF16, tag="ztsb")
                nc.scalar.activation(zt_sb[:], zt_ps[:, 0:128], AF.Identity,
                                     bias=b3c_sb[:, 0:1])
                o_ps = tps.tile([128, HC], F32, tag="tabps")
                nc.tensor.matmul(o_ps[:, 0:OUT_F], zt_sb[:], w4_sb[:],
                                 start=True, stop=True)
                o_pre = fpool.tile([128, OUT_F], F32, tag="opre")
                nc.vector.scalar_tensor_tensor(
                    out=o_pre[:], in0=o_ps[:, 0:OUT_F], scalar=1.0,
                    in1=b4f_sb[:], op0=OP.mult, op1=OP.add)
                o_sb = fpool.tile([128, OUT_F], F32, tag="osb")
                nc.scalar.activation(o_sb[:], o_pre[:], AF.Sigmoid)
                nc.sync.dma_start(out=out_ext[t * 128:(t + 1) * 128, :],
                                  in_=o_sb[:])

            # ================= phase schedule =================
            import os as _os
            _upto = int(_os.environ.get("KPHASES", "9"))

            table_local_l1()
            if _upto >= 2:
                ag2 = lambda t: ag_fire((xl_own2a, xl_own2b),
                                        (xl_all2a, xl_all2b), t)
                conv_layer(xl_all1a, xl_all1b, loc1, LR1, fin1, y_act=4,
                           ag_cb=ag2)
            if _upto >= 4:
                conv_layer(xl_all2a, xl_all2b, loc2, LR2, fin2, y_act=4)
            else:
                zt = fpool.tile([128, OUT_F], F32, tag="osb")
                nc.vector.memset(zt[:], 0.0)
                for t in range(NTILES):
                    nc.sync.dma_start(out=out_ext[t * 128:(t + 1) * 128, :],
                                      in_=zt[:])

    nc.compile()
    return nc


# ---------------------------------------------------------------- entry point
def kernel(**inputs):
    from concourse import bass_utils

    src = np.asarray(inputs["edge_index"][0], np.int64)
    dst = np.asarray(inputs["edge_index"][1], np.int64)
    x = np.asarray(inputs["x"], np.float32)

    pack = _pack_graph(src, dst)
    nos = pack["node_of_slot"]
    valid = nos >= 0
    x_slot = np.zeros((S, IN_F), np.float32)
    x_slot[valid] = x[nos[valid]]

    def bf(a):
        return np.ascontiguousarray(np.asarray(a, np.float32)).astype(BF)

    # --- per-head column permutation (+att cols first) + pre-scale by att
    def prep_layer(att):
        att = np.asarray(att, np.float32).reshape(2, 128)
        perm = np.zeros(HC, np.int64)
        ranges = []
        for h in range(2):
            a = att[h]
            pos = np.where(a > 0)[0]
            neg = np.where(a <= 0)[0]
            perm[h * 128:(h + 1) * 128] = h * 128 + np.concatenate([pos, neg])
            p = len(pos)
            if p:
                ranges.append((h * 128, h * 128 + p, "max"))
            if p < 128:
                ranges.append((h * 128 + p, (h + 1) * 128, "min"))
        att_p = att.reshape(HC)[perm]
        att_p = np.where(np.abs(att_p) < 1e-30, 1e-30, att_p)
        return perm, att_p, ranges

    perm1, att1p, LR1 = prep_layer(inputs["att1"])
    perm2, att2p, LR2 = prep_layer(inputs["att2"])
    _LR_RANGES["l1"] = LR1
    _LR_RANGES["l2"] = LR2

    Wl1p = np.asarray(inputs["Wl1"], np.float32)[:, perm1] * att1p[None, :]
    Wr1p = np.asarray(inputs["Wr1"], np.float32)[:, perm1] * att1p[None, :]
    Wl2p = (np.asarray(inputs["Wl2"], np.float32)[perm1][:, perm2]
            * att2p[None, :])
    Wr2p = (np.asarray(inputs["Wr2"], np.float32)[perm1][:, perm2]
            * att2p[None, :])
    W3p = np.asarray(inputs["W3"], np.float32)[perm2]
    b1p = np.asarray(inputs["b1"], np.float32)[perm1]
    b2p = np.asarray(inputs["b2"], np.float32)[perm2]

    common = {
        "wl1": bf(Wl1p), "wr1": bf(Wr1p),
        "wl2": bf(Wl2p), "wr2": bf(Wr2p),
        "w3": bf(W3p), "w4": bf(inputs["W4"]),
        "iavT1": np.ascontiguousarray(
            (1.0 / att1p).reshape(2, 128).T.astype(np.float32)),
        "iavT2": np.ascontiguousarray(
            (1.0 / att2p).reshape(2, 128).T.astype(np.float32)),
        "bT1": np.ascontiguousarray(b1p.reshape(2, 128).T.astype(np.float32)),
        "bT2": np.ascontiguousarray(b2p.reshape(2, 128).T.astype(np.float32)),
        "b3c": np.asarray(inputs["b3"], np.float32).reshape(128, 1),
        "b4f": np.tile(np.asarray(inputs["b4"], np.float32)[None, :], (128, 1)),
        "idenBF": np.eye(128, dtype=np.float32).astype(BF),
        "epsc": np.full((128, 1), 1e-16, np.float32),
    }

    in_maps = []
    for k in range(NCORES):
        m = dict(common)
        m["xoT"] = np.ascontiguousarray(
            x_slot[k * SPC:(k + 1) * SPC].T).astype(BF)
        ixla = np.empty((NTILES, 128, TA * 8), np.int16)
        ixlb = np.empty((NTILES, 128, TB * 8), np.int16)
        for t in range(NTILES):
            ixla[t] = _wrap_idx(pack["idxXL"][k, t, :TA * 128])
            ixlb[t] = _wrap_idx(pack["idxXL"][k, t, TA * 128:])
        m["idxXLA"] = ixla
        m["idxXLB"] = ixlb
        m["mkd"] = pack["mk"][k].astype(BF)
        m["mkTd"] = pack["mkT"][k].astype(BF)
        in_maps.append(m)

    if "nc" not in _NC_CACHE:
        _NC_CACHE["nc"] = _build_nc()
    nc = _NC_CACHE["nc"]

    res = bass_utils.run_bass_kernel_spmd(nc, in_maps,
                                          core_ids=list(range(NCORES)),
                                          **_RUN_OPTS)
    _LAST_RESULTS["res"] = res
    out_slots = np.concatenate([res.results[k]["out"] for k in range(NCORES)], 0)
    return out_slots[pack["slot_of_node"]].astype(np.float32)
